# revision 2
# baseline (speedup 1.0000x reference)
import numpy as np
import jax
import jax.numpy as jnp
from jax.sharding import Mesh, NamedSharding, PartitionSpec as P

try:
    from jax.experimental.shard_map import shard_map
except ImportError:
    from jax import shard_map

# nn_GTN_58205396795517: 2-layer TransformerConv GNN
N = 100000
E = 800000
D = 64
H = 4
C = 64
M = 8

_INV_SQRT_C = np.float32(1.0 / np.sqrt(C))

_state = None


def _build():
    """Compile the pipeline as many small jits — the neuronx-cc backend
    crashes (DataLocalityOpt assert) on fused gather+arith graphs, so each
    stage is kept to a single primitive pattern known to compile."""
    global _state
    if _state is not None:
        return _state
    mesh = Mesh(np.array(jax.devices()[:M]), ('x',))
    rep = NamedSharding(mesh, P())
    esh = NamedSharding(mesh, P('x'))

    def smap(fn, in_specs, out_specs):
        return jax.jit(shard_map(fn, mesh=mesh, in_specs=in_specs,
                                 out_specs=out_specs))

    J = {}
    # dense projections (replicated on every core)
    J['dense'] = smap(lambda x, Wqkv, bqkv, Ws, bs:
                      tuple(jnp.split(x @ Wqkv + bqkv, 3, axis=1))
                      + (x @ Ws + bs,),
                      (P(),) * 5, (P(), P(), P(), P()))
    # row gathers (edge-sharded output); jnp.take compiles where t[i] hits
    # a neuronx-cc DataLocalityOpt assert
    J['gather'] = smap(lambda t, i: jnp.take(t, i, axis=0),
                       (P(), P('x')), P('x'))
    # per-edge head-wise dot product
    J['dot'] = smap(lambda a, b:
                    (a * b).reshape(-1, H, C).sum(-1) * _INV_SQRT_C,
                    (P('x'), P('x')), P('x'))
    # exp (softmax without max-shift: logits are O(1), shift-invariant)
    J['exp'] = smap(lambda a: jnp.exp(a), (P('x'),), P('x'))
    # partial segment sum + all-reduce -> replicated node-indexed buffer
    J['segsum'] = smap(lambda v, i: jax.lax.psum(
        jax.ops.segment_sum(v, i, num_segments=N), 'x'),
        (P('x'), P('x')), P())
    # attn = ex / denom[dst]
    J['norm'] = smap(lambda ex, denom, i: ex / (denom[i] + 1e-16),
                     (P('x'), P(), P('x')), P('x'))
    # msg = v[src] * attn (attn broadcast across C within each head)
    J['msg'] = smap(lambda vs, at: vs * jnp.repeat(at, C, axis=1),
                    (P('x'), P('x')), P('x'))
    # head mean + skip connection
    J['out'] = smap(lambda agg, skip:
                    agg.reshape(N, H, C).mean(axis=1) + skip,
                    (P(), P()), P())
    J['relu'] = smap(lambda h: jax.nn.relu(h), (P(),), P())

    _state = (mesh, rep, esh, J)
    return _state


def _layer(x_d, src, dst, Wqkv, bqkv, Ws, bs, J):
    q, k, v, skip = J['dense'](x_d, Wqkv, bqkv, Ws, bs)
    qd = J['gather'](q, dst)
    ks = J['gather'](k, src)
    alpha = J['dot'](qd, ks)
    ex = J['exp'](alpha)
    denom = J['segsum'](ex, dst)
    attn = J['norm'](ex, denom, dst)
    vs = J['gather'](v, src)
    msg = J['msg'](vs, attn)
    agg = J['segsum'](msg, dst)
    return J['out'](agg, skip)


def kernel(x, edge_index, Wq1, bq1, Wk1, bk1, Wv1, bv1, Ws1, bs1,
           Wq2, bq2, Wk2, bk2, Wv2, bv2, Ws2, bs2):
    mesh, rep, esh, J = _build()

    ei = np.asarray(edge_index)
    src = jax.device_put(jnp.asarray(ei[0]), esh)
    dst = jax.device_put(jnp.asarray(ei[1]), esh)

    def prep(Wq, Wk, Wv, bq, bk, bv):
        Wqkv = np.concatenate([np.asarray(Wq), np.asarray(Wk),
                               np.asarray(Wv)], axis=1)
        bqkv = np.concatenate([np.asarray(bq), np.asarray(bk),
                               np.asarray(bv)])
        return (jax.device_put(jnp.asarray(Wqkv), rep),
                jax.device_put(jnp.asarray(bqkv), rep))

    W1, b1 = prep(Wq1, Wk1, Wv1, bq1, bk1, bv1)
    W2, b2 = prep(Wq2, Wk2, Wv2, bq2, bk2, bv2)
    pr = lambda a: jax.device_put(jnp.asarray(np.asarray(a)), rep)
    Ws1d, bs1d, Ws2d, bs2d = pr(Ws1), pr(bs1), pr(Ws2), pr(bs2)

    x_d = pr(x)
    h = _layer(x_d, src, dst, W1, b1, Ws1d, bs1d, J)
    h = J['relu'](h)
    out = _layer(h, src, dst, W2, b2, Ws2d, bs2d, J)
    return np.asarray(jax.device_get(out)).astype(np.float32)



# revision 8
# speedup vs baseline: 1.3213x; 1.3213x over previous
"""nn_GTN_58205396795517: 2-layer TransformerConv GNN on 8 NeuronCores.

Bass/Tile kernel. Strategy:
  - Destination-shard nodes across the 8 cores (12544 nodes/core, padded
    N=100352). Each core owns all edges into its node range; no cross-core
    reduction is needed for the segment softmax / scatter-add.
  - Per core, edges are sorted by destination and packed into 98 windows of
    128 consecutive destination nodes; each window holds a fixed number of
    128-edge tiles (capacity derived from the actual graph).
  - Attention logits use the fused bilinear form
        alpha[e,h] = x_dst . (Wk_h Wq_h^T x_src) + bq_h . (Wk_h^T x_src)
    (destination-only bias terms cancel in the segment softmax), so only
    x rows are ever gathered: 288 B per edge endpoint via indirect DMA.
  - Per tile: one-hot slot matrix (iota compare) + PE matmul performs the
    in-window segment reduction straight into PSUM; window finalize divides
    by the accumulated denominators, head-averages, and adds skip in place.
  - h is AllGathered between the layers inside the same NEFF.

Falls back to a pure-JAX shard_map implementation if the Bass path fails.
"""
import numpy as np

NC = 8
N = 100000
SH = 12544            # nodes per core (128-aligned)
NPAD = SH * NC        # 100352
D = 64
H = 4
ROWW = 72             # padded node-table row width (64 feat | 1.0 | pad)
WIN = SH // 128       # 98 windows per core
AW = 65               # augmented per-head width in the k' table (64 + bias col)

_state = {}


# ----------------------------------------------------------------------------
# walrus workarounds (wait-split + Tile tail drain)
# ----------------------------------------------------------------------------
def _install_patches():
    import json as _json
    import os as _os
    from concourse import bass_utils, bass2jax, tile
    from concourse.vector_clock import ScopedClock

    if getattr(bass_utils, "_gtn_patch_installed", False):
        return
    bass_utils._gtn_patch_installed = True

    MAXW = 1

    def _split_waits(bir):
        changed = False
        for fn in bir.get("functions", []):
            for bb in fn.get("blocks", []):
                insts = bb.get("instructions")
                if not insts:
                    continue
                out = []
                for inst in insts:
                    si = inst.get("sync_info") or {}
                    waits = si.get("on_wait") or []
                    if len(waits) > MAXW:
                        changed = True
                        extra, keep = waits[:-MAXW], waits[-MAXW:]
                        for n, i0 in enumerate(range(0, len(extra), MAXW)):
                            out.append({
                                "name": f"{inst['name']}_wsplit{n}",
                                "opcode": "NoOp",
                                "engine": inst.get("engine"),
                                "ins": [], "outs": [],
                                "sync_info": {"on_wait": extra[i0:i0 + MAXW],
                                              "on_update": []},
                            })
                        si = dict(si); si["on_wait"] = keep
                        inst = dict(inst); inst["sync_info"] = si
                    out.append(inst)
                bb["instructions"] = out
        return changed

    orig = bass_utils.compile_bir_kernel

    def patched(bir_json, tmpdir, neff_name="file.neff"):
        bir = _json.loads(bir_json)
        if _split_waits(bir):
            bir_json = _json.dumps(bir).encode()
        return orig(bir_json, tmpdir, neff_name=neff_name)

    bass_utils.compile_bir_kernel = patched
    bass2jax.compile_bir_kernel = patched

    def _drain_and_barrier(self, tick_clock, wait_clock):
        nop_inst = self.nc.sync.nop(nofuse=True)
        wait_clock.add_sem_waits(
            nop_inst.ins, ScopedClock({None: tick_clock.global_clock}))
        self.nc.all_engine_barrier()
        assert self.sems is not None
        popped = self.nc._tile_sem_poison_stack.pop()
        assert popped is self._sem_poison
        self.nc.clear_and_free_semaphores(list(self.sems.allocated().values()))
        self.nc.all_engine_barrier()

    tile.TileContext._drain_and_barrier = _drain_and_barrier


# ----------------------------------------------------------------------------
# host-side preprocessing
# ----------------------------------------------------------------------------
def _prep_edges(edge_index):
    src = np.asarray(edge_index[0], np.int64)
    dst = np.asarray(edge_index[1], np.int64)
    per_core = []
    t_w = 1
    for c in range(NC):
        lo = c * SH
        m = (dst >= lo) & (dst < lo + SH)
        s = src[m]
        d = dst[m] - lo
        order = np.argsort(d, kind="stable")
        s, d = s[order], d[order]
        w = d >> 7
        cnt = np.bincount(w, minlength=WIN).astype(np.int64)
        if len(s):
            t_w = max(t_w, int(np.ceil(cnt.max() / 128)))
        per_core.append((s, d, cnt))
    cols = WIN * t_w
    idx_src = np.zeros((NC, 128, cols), np.int32)
    idx_dst = np.zeros((NC, 128, cols), np.int32)
    slot = np.full((NC, 128, cols), -1.0, np.float32)
    for c, (s, d, cnt) in enumerate(per_core):
        start = 0
        for win in range(WIN):
            n = int(cnt[win])
            ss = s[start:start + n]
            dd = d[start:start + n]
            for t in range(int(np.ceil(n / 128))):
                a, b = t * 128, min((t + 1) * 128, n)
                col = win * t_w + t
                idx_src[c, 0:b - a, col] = ss[a:b]
                idx_dst[c, 0:b - a, col] = dd[a:b]
                slot[c, 0:b - a, col] = (dd[a:b] - 128 * win).astype(np.float32)
            start += n
    return idx_src, idx_dst, slot, t_w


def _prep_weights(Wq, bq, Wk, bk, Wv, bv, Ws, bs):
    Wq = np.asarray(Wq, np.float32); Wk = np.asarray(Wk, np.float32)
    Wv = np.asarray(Wv, np.float32); Ws = np.asarray(Ws, np.float32)
    bq = np.asarray(bq, np.float32); bv = np.asarray(bv, np.float32)
    bs = np.asarray(bs, np.float32)
    wkqs = np.zeros((AW, H * AW), np.float32)
    for h in range(H):
        Wq_h = Wq[:, h * D:(h + 1) * D]
        Wk_h = Wk[:, h * D:(h + 1) * D]
        wkqs[0:D, h * AW:h * AW + D] = Wk_h @ Wq_h.T
        wkqs[0:D, h * AW + D] = Wk_h @ bq[h * D:(h + 1) * D]
    wv_aug = np.concatenate([Wv, bv[None, :]], axis=0)      # [65, 256]
    ws_aug = np.concatenate([Ws, bs[None, :]], axis=0)      # [65, 64]
    return wkqs, wv_aug, ws_aug


# ----------------------------------------------------------------------------
# Bass program
# ----------------------------------------------------------------------------
def _build_nc(t_w):
    import concourse.bass as bass
    import concourse.mybir as mybir
    import concourse.tile as tile
    from concourse.masks import make_identity

    f32 = mybir.dt.float32
    i32 = mybir.dt.int32
    COLS = WIN * t_w

    nc = bass.Bass()
    xsh = nc.dram_tensor("xsh", [SH, ROWW], f32, kind="ExternalInput")
    isrc = nc.dram_tensor("isrc", [128, COLS], i32, kind="ExternalInput")
    idst = nc.dram_tensor("idst", [128, COLS], i32, kind="ExternalInput")
    slot = nc.dram_tensor("slot", [128, COLS], f32, kind="ExternalInput")
    wk1 = nc.dram_tensor("wk1", [AW, H * AW], f32, kind="ExternalInput")
    wv1 = nc.dram_tensor("wv1", [AW, H * D], f32, kind="ExternalInput")
    ws1 = nc.dram_tensor("ws1", [AW, D], f32, kind="ExternalInput")
    wk2 = nc.dram_tensor("wk2", [AW, H * AW], f32, kind="ExternalInput")
    wv2 = nc.dram_tensor("wv2", [AW, H * D], f32, kind="ExternalInput")
    ws2 = nc.dram_tensor("ws2", [AW, D], f32, kind="ExternalInput")
    iota = nc.dram_tensor("iota", [128, 128], f32, kind="ExternalInput")
    out_sh = nc.dram_tensor("out_sh", [SH, D], f32, kind="ExternalOutput")

    xbounce = nc.dram_tensor("xbounce", [SH, ROWW], f32)
    xfull = nc.dram_tensor("xfull", [NPAD, ROWW], f32, addr_space="Shared")
    hown = nc.dram_tensor("hown", [SH, ROWW], f32)
    hbounce = nc.dram_tensor("hbounce", [SH, ROWW], f32)
    hfull = nc.dram_tensor("hfull", [NPAD, ROWW], f32, addr_space="Shared")

    groups = [list(range(NC))]

    with tile.TileContext(nc) as tc:
        with tc.tile_pool(name="const", bufs=1) as cp, \
             tc.tile_pool(name="idxp", bufs=1) as ip, \
             tc.tile_pool(name="work", bufs=3) as wp, \
             tc.tile_pool(name="winp", bufs=2) as wnp, \
             tc.tile_pool(name="pmmA", bufs=2, space="PSUM") as pA, \
             tc.tile_pool(name="pmmB", bufs=2, space="PSUM") as pB, \
             tc.tile_pool(name="ptr", bufs=1, space="PSUM") as pT, \
             tc.tile_pool(name="pwin", bufs=2, space="PSUM") as pW, \
             tc.tile_pool(name="pskip", bufs=1, space="PSUM") as pS:

            ident = cp.tile([128, 128], f32)
            make_identity(nc, ident[:])
            iota_t = cp.tile([128, 128], f32)
            nc.sync.dma_start(out=iota_t[:], in_=iota[:])

            w_k = {}
            w_v = {}
            w_s = {}
            for L, (wk, wv, ws) in ((1, (wk1, wv1, ws1)),
                                    (2, (wk2, wv2, ws2))):
                w_k[L] = cp.tile([AW, H * AW], f32, tag=f"wk{L}", name=f"wk{L}_t")
                nc.sync.dma_start(out=w_k[L][:], in_=wk[:])
                w_v[L] = cp.tile([AW, H * D], f32, tag=f"wv{L}", name=f"wv{L}_t")
                nc.sync.dma_start(out=w_v[L][:], in_=wv[:])
                w_s[L] = cp.tile([AW, D], f32, tag=f"ws{L}", name=f"ws{L}_t")
                nc.sync.dma_start(out=w_s[L][:], in_=ws[:])

            isrc_t = ip.tile([128, COLS], i32)
            nc.sync.dma_start(out=isrc_t[:], in_=isrc[:])
            idst_t = ip.tile([128, COLS], i32)
            nc.sync.dma_start(out=idst_t[:], in_=idst[:])
            slot_t = ip.tile([128, COLS], f32)
            nc.sync.dma_start(out=slot_t[:], in_=slot[:])

            # x -> internal bounce -> allgather to the full table
            nc.gpsimd.dma_start(out=xbounce[:], in_=xsh[:])
            nc.gpsimd.collective_compute(
                "AllGather", mybir.AluOpType.bypass, replica_groups=groups,
                ins=[xbounce[:]], outs=[xfull[:]])

            def layer(L, table, own, relu, h_out):
                """One TransformerConv layer."""
                for w in range(WIN):
                    psw = pW.tile([128, H * AW], f32, tag="pw")
                    for t in range(t_w):
                        col = w * t_w + t
                        xs = wp.tile([128, ROWW], f32, tag="xs")
                        nc.gpsimd.indirect_dma_start(
                            out=xs[:], out_offset=None, in_=table[:],
                            in_offset=bass.IndirectOffsetOnAxis(
                                ap=isrc_t[:, col:col + 1], axis=0))
                        xd = wp.tile([128, ROWW], f32, tag="xd")
                        nc.gpsimd.indirect_dma_start(
                            out=xd[:], out_offset=None, in_=own[:],
                            in_offset=bass.IndirectOffsetOnAxis(
                                ap=idst_t[:, col:col + 1], axis=0))
                        # transpose x_src features
                        pst = pT.tile([AW, 128], f32, tag="pt")
                        nc.tensor.transpose(out=pst[:], in_=xs[:, 0:AW],
                                            identity=ident[:])
                        xsT = wp.tile([AW, 128], f32, tag="xsT")
                        nc.scalar.activation(
                            out=xsT[:], in_=pst[:],
                            func=mybir.ActivationFunctionType.Copy)
                        # k' | s3 and v
                        psa = pA.tile([128, H * AW], f32, tag="pa")
                        nc.tensor.matmul(out=psa[:], lhsT=xsT[:],
                                         rhs=w_k[L][:], start=True, stop=True)
                        psb = pB.tile([128, H * D], f32, tag="pb")
                        nc.tensor.matmul(out=psb[:], lhsT=xsT[:],
                                         rhs=w_v[L][:], start=True, stop=True)
                        # alpha = sum_c k'(aug) * x_dst(aug)
                        tmp = wp.tile([128, H * AW], f32, tag="tmp")
                        nc.vector.tensor_tensor(
                            out=tmp[:].rearrange("p (h c) -> p h c", c=AW),
                            in0=psa[:].rearrange("p (h c) -> p h c", c=AW),
                            in1=xd[:, 0:AW].rearrange("p (o c) -> p o c", o=1)
                                .to_broadcast([128, H, AW]),
                            op=mybir.AluOpType.mult)
                        alpha = wp.tile([128, H], f32, tag="alpha")
                        nc.vector.tensor_reduce(
                            out=alpha[:],
                            in_=tmp[:].rearrange("p (h c) -> p h c", c=AW),
                            axis=mybir.AxisListType.X, op=mybir.AluOpType.add)
                        ex = wp.tile([128, H], f32, tag="ex")
                        nc.scalar.activation(
                            out=ex[:], in_=alpha[:],
                            func=mybir.ActivationFunctionType.Exp, scale=0.125)
                        # stg = [ ex*v | ex ]
                        stg = wp.tile([128, H * AW], f32, tag="stg")
                        nc.vector.tensor_tensor(
                            out=stg[:, 0:H * D].rearrange("p (h c) -> p h c", c=D),
                            in0=psb[:].rearrange("p (h c) -> p h c", c=D),
                            in1=ex[:].rearrange("p (h o) -> p h o", o=1)
                                .to_broadcast([128, H, D]),
                            op=mybir.AluOpType.mult)
                        nc.vector.tensor_copy(out=stg[:, H * D:H * AW], in_=ex[:])
                        # one-hot slot matrix
                        smat = wp.tile([128, 128], f32, tag="smat")
                        nc.vector.tensor_tensor(
                            out=smat[:],
                            in0=slot_t[:, col:col + 1].to_broadcast([128, 128]),
                            in1=iota_t[:], op=mybir.AluOpType.is_equal)
                        # segment-reduce into the window accumulator
                        nc.tensor.matmul(out=psw[:], lhsT=smat[:], rhs=stg[:],
                                         start=(t == 0), stop=(t == t_w - 1))

                    # ---- window finalize ----
                    xow = wnp.tile([128, ROWW], f32, tag="xow")
                    nc.sync.dma_start(out=xow[:], in_=own[w * 128:(w + 1) * 128, :])
                    pst2 = pT.tile([AW, 128], f32, tag="pt")
                    nc.tensor.transpose(out=pst2[:], in_=xow[:, 0:AW],
                                        identity=ident[:])
                    xowT = wnp.tile([AW, 128], f32, tag="xowT")
                    nc.scalar.activation(out=xowT[:], in_=pst2[:],
                                         func=mybir.ActivationFunctionType.Copy)
                    pss = pS.tile([128, D], f32, tag="ps")
                    nc.tensor.matmul(out=pss[:], lhsT=xowT[:], rhs=w_s[L][:],
                                     start=True, stop=True)
                    # rd = 1 / (4*denom + 4e-16)
                    rdin = wnp.tile([128, H], f32, tag="rdin")
                    nc.scalar.activation(
                        out=rdin[:], in_=psw[:, H * D:H * AW],
                        func=mybir.ActivationFunctionType.Copy,
                        scale=4.0, bias=4e-16)
                    rd = wnp.tile([128, H], f32, tag="rd")
                    nc.vector.reciprocal(out=rd[:], in_=rdin[:])
                    wtmp = wnp.tile([128, H * D], f32, tag="wtmp")
                    nc.vector.tensor_tensor(
                        out=wtmp[:].rearrange("p (h c) -> p h c", c=D),
                        in0=psw[:, 0:H * D].rearrange("p (h c) -> p h c", c=D),
                        in1=rd[:].rearrange("p (h o) -> p h o", o=1)
                            .to_broadcast([128, H, D]),
                        op=mybir.AluOpType.mult)
                    hpart = wnp.tile([128, D], f32, tag="hpart")
                    nc.vector.tensor_reduce(
                        out=hpart[:],
                        in_=wtmp[:].rearrange("p (h c) -> p c h", c=D),
                        axis=mybir.AxisListType.X, op=mybir.AluOpType.add)
                    hsum = wnp.tile([128, ROWW], f32, tag="hsum")
                    nc.vector.tensor_add(out=hsum[:, 0:D], in0=hpart[:],
                                         in1=pss[:])
                    if relu:
                        nc.scalar.activation(
                            out=hsum[:, 0:D], in_=hsum[:, 0:D],
                            func=mybir.ActivationFunctionType.Relu)
                    if h_out is None:
                        nc.sync.dma_start(
                            out=out_sh[w * 128:(w + 1) * 128, :],
                            in_=hsum[:, 0:D])
                    else:
                        nc.gpsimd.memset(hsum[:, D + 1:ROWW], 0.0)
                        nc.vector.tensor_copy(out=hsum[:, D:D + 1],
                                              in_=iota_t[:, 1:2])
                        nc.sync.dma_start(
                            out=hown[w * 128:(w + 1) * 128, :], in_=hsum[:])
                        nc.sync.dma_start(
                            out=hbounce[w * 128:(w + 1) * 128, :], in_=hsum[:])

            layer(1, xfull, xsh, relu=True, h_out=True)
            nc.gpsimd.collective_compute(
                "AllGather", mybir.AluOpType.bypass, replica_groups=groups,
                ins=[hbounce[:]], outs=[hfull[:]])
            layer(2, hfull, hown, relu=False, h_out=None)

    return nc


# ----------------------------------------------------------------------------
# SPMD runner (cached jitted executable)
# ----------------------------------------------------------------------------
class _Runner:
    def __init__(self, nc, donate=True):
        import jax
        from jax.sharding import Mesh, PartitionSpec, NamedSharding
        try:
            from jax.experimental.shard_map import shard_map
        except ImportError:
            from jax import shard_map
        from concourse.bass2jax import (_bass_exec_p, install_neuronx_cc_hook,
                                        partition_id_tensor)
        import concourse.mybir as mybir

        install_neuronx_cc_hook()
        self.jax = jax
        partition_name = (nc.partition_id_tensor.name
                          if nc.partition_id_tensor else None)
        in_names, out_names, out_avals = [], [], []
        zero_outs = []
        for alloc in nc.m.functions[0].allocations:
            if not isinstance(alloc, mybir.MemoryLocationSet):
                continue
            name = alloc.memorylocations[0].name
            if alloc.kind == "ExternalInput":
                if name != partition_name:
                    in_names.append(name)
            elif alloc.kind == "ExternalOutput":
                shape = tuple(alloc.tensor_shape)
                dtype = mybir.dt.np(alloc.dtype)
                out_names.append(name)
                out_avals.append(jax.core.ShapedArray(shape, dtype))
                zero_outs.append(np.zeros(shape, dtype))
        self.in_names, self.out_names = in_names, out_names
        self.out_avals, self.zero_outs = out_avals, zero_outs
        n_params, n_outs = len(in_names), len(out_names)
        all_in = in_names + out_names + ([partition_name] if partition_name else [])

        def _body(*args):
            operands = list(args)
            if partition_name is not None:
                operands.append(partition_id_tensor())
            return tuple(_bass_exec_p.bind(
                *operands, out_avals=tuple(out_avals), in_names=tuple(all_in),
                out_names=tuple(out_names), lowering_input_output_aliases=(),
                sim_require_finite=False, sim_require_nnan=False, nc=nc))

        devices = jax.devices()[:NC]
        self.mesh = Mesh(np.asarray(devices), ("core",))
        self.sh = NamedSharding(self.mesh, PartitionSpec("core"))
        kwargs = dict(keep_unused=True)
        if donate:
            kwargs["donate_argnums"] = tuple(range(n_params, n_params + n_outs))
        self.donate = donate
        self.fn = jax.jit(shard_map(
            _body, mesh=self.mesh,
            in_specs=(PartitionSpec("core"),) * (n_params + n_outs),
            out_specs=(PartitionSpec("core"),) * n_outs, check_rep=False),
            **kwargs)
        self.n_params = n_params

    def device_inputs(self, in_maps):
        concat = [
            np.concatenate([np.asarray(m[name]) for m in in_maps], axis=0)
            for name in self.in_names
        ]
        return [self.jax.device_put(a, self.sh) for a in concat]

    def zeros(self):
        return [self.jax.device_put(
            np.zeros((NC * z.shape[0], *z.shape[1:]), z.dtype), self.sh)
            for z in self.zero_outs]

    def run(self, dev_inputs):
        outs = self.fn(*dev_inputs, *self.zeros())
        return [np.asarray(o) for o in outs]


# ----------------------------------------------------------------------------
# public entry
# ----------------------------------------------------------------------------
def _prepare(x, edge_index, weights):
    """Returns (runner, in_maps)."""
    idx_src, idx_dst, slot, t_w = _prep_edges(edge_index)
    key = ("bass", t_w)
    if key not in _state:
        _install_patches()
        nc = _build_nc(t_w)
        _state[key] = _Runner(nc)
    runner = _state[key]

    x = np.asarray(x, np.float32)
    xpad = np.zeros((NPAD, ROWW), np.float32)
    xpad[:N, 0:D] = x
    xpad[:, D] = 1.0

    wk1, wv1, ws1 = _prep_weights(*weights[0])
    wk2, wv2, ws2 = _prep_weights(*weights[1])
    iota = np.broadcast_to(np.arange(128, dtype=np.float32), (128, 128)).copy()

    in_maps = []
    for c in range(NC):
        in_maps.append({
            "xsh": xpad[c * SH:(c + 1) * SH],
            "isrc": idx_src[c], "idst": idx_dst[c], "slot": slot[c],
            "wk1": wk1, "wv1": wv1, "ws1": ws1,
            "wk2": wk2, "wv2": wv2, "ws2": ws2,
            "iota": iota,
        })
    return runner, in_maps


def _kernel_bass(x, edge_index, weights):
    runner, in_maps = _prepare(x, edge_index, weights)
    dev_in = runner.device_inputs(in_maps)
    outs = runner.run(dev_in)
    full = outs[0].reshape(NC * SH, D)
    return full[:N].copy()


# ---------------------------- JAX fallback ----------------------------------
def _kernel_jax(x, edge_index, weights):
    import jax
    import jax.numpy as jnp
    from jax.sharding import Mesh, NamedSharding, PartitionSpec as P
    try:
        from jax.experimental.shard_map import shard_map
    except ImportError:
        from jax import shard_map

    E = edge_index.shape[1]
    M = NC
    mesh = Mesh(np.array(jax.devices()[:M]), ('x',))
    rep = NamedSharding(mesh, P())
    esh = NamedSharding(mesh, P('x'))
    inv = np.float32(1.0 / np.sqrt(D))

    def smap(fn, in_specs, out_specs):
        return jax.jit(shard_map(fn, mesh=mesh, in_specs=in_specs,
                                 out_specs=out_specs))

    J = {}
    J['dense'] = smap(lambda x_, Wqkv, bqkv, Ws, bs:
                      tuple(jnp.split(x_ @ Wqkv + bqkv, 3, axis=1))
                      + (x_ @ Ws + bs,), (P(),) * 5, (P(), P(), P(), P()))
    J['gather'] = smap(lambda t, i: jnp.take(t, i, axis=0),
                       (P(), P('x')), P('x'))
    J['dot'] = smap(lambda a, b: (a * b).reshape(-1, H, D).sum(-1) * inv,
                    (P('x'), P('x')), P('x'))
    J['exp'] = smap(lambda a: jnp.exp(a), (P('x'),), P('x'))
    J['segsum'] = smap(lambda v, i: jax.lax.psum(
        jax.ops.segment_sum(v, i, num_segments=N), 'x'),
        (P('x'), P('x')), P())
    J['norm'] = smap(lambda ex, den, i: ex / (den[i] + 1e-16),
                     (P('x'), P(), P('x')), P('x'))
    J['msg'] = smap(lambda vs, at: vs * jnp.repeat(at, D, axis=1),
                    (P('x'), P('x')), P('x'))
    J['out'] = smap(lambda agg, skip: agg.reshape(N, H, D).mean(axis=1) + skip,
                    (P(), P()), P())
    J['relu'] = smap(lambda h: jax.nn.relu(h), (P(),), P())

    def lyr(x_d, s, d, Wqkv, bqkv, Ws, bs):
        q, k, v, skip = J['dense'](x_d, Wqkv, bqkv, Ws, bs)
        alpha = J['dot'](J['gather'](q, d), J['gather'](k, s))
        ex = J['exp'](alpha)
        den = J['segsum'](ex, d)
        attn = J['norm'](ex, den, d)
        msg = J['msg'](J['gather'](v, s), attn)
        return J['out'](J['segsum'](msg, d), skip)

    ei = np.asarray(edge_index)
    s = jax.device_put(jnp.asarray(ei[0]), esh)
    d = jax.device_put(jnp.asarray(ei[1]), esh)

    def prep(Wq, bq, Wk, bk, Wv, bv, Ws, bs):
        Wqkv = np.concatenate([Wq, Wk, Wv], axis=1)
        bqkv = np.concatenate([bq, bk, bv])
        return (jax.device_put(jnp.asarray(Wqkv), rep),
                jax.device_put(jnp.asarray(bqkv), rep),
                jax.device_put(jnp.asarray(Ws), rep),
                jax.device_put(jnp.asarray(bs), rep))

    W1 = prep(*weights[0])
    W2 = prep(*weights[1])
    x_d = jax.device_put(jnp.asarray(np.asarray(x)), rep)
    h = lyr(x_d, s, d, *W1)
    h = J['relu'](h)
    out = lyr(h, s, d, *W2)
    return np.asarray(jax.device_get(out)).astype(np.float32)


def kernel(x, edge_index, Wq1, bq1, Wk1, bk1, Wv1, bv1, Ws1, bs1,
           Wq2, bq2, Wk2, bk2, Wv2, bv2, Ws2, bs2):
    weights = ((Wq1, bq1, Wk1, bk1, Wv1, bv1, Ws1, bs1),
               (Wq2, bq2, Wk2, bk2, Wv2, bv2, Ws2, bs2))
    edge_index = np.asarray(edge_index)
    try:
        return _kernel_bass(np.asarray(x), edge_index, weights)
    except Exception as e:  # pragma: no cover - safety net
        import traceback
        traceback.print_exc()
        print(f"[kernel] bass path failed ({e!r}); falling back to JAX")
        return _kernel_jax(np.asarray(x), edge_index, weights)


# revision 18
# speedup vs baseline: 3.5646x; 2.6978x over previous
"""nn_GTN_58205396795517: 2-layer TransformerConv GNN on 8 NeuronCores.

Bass/Tile kernel. Strategy:
  - Destination-shard nodes across the 8 cores (12544 nodes/core, padded
    N=100352). Each core owns all edges into its node range; no cross-core
    reduction is needed for the segment softmax / scatter-add.
  - Per core, edges are sorted by destination and packed into 98 windows of
    128 consecutive destination nodes; each window holds a fixed number of
    128-edge tiles (capacity derived from the actual graph).
  - Attention logits use the fused bilinear form
        alpha[e,h] = x_dst . (Wk_h Wq_h^T x_src) + bq_h . (Wk_h^T x_src)
    (destination-only bias terms cancel in the segment softmax), so only
    x rows are ever gathered: 288 B per edge endpoint via indirect DMA.
  - Per tile: one-hot slot matrix (iota compare) + PE matmul performs the
    in-window segment reduction straight into PSUM; window finalize divides
    by the accumulated denominators, head-averages, and adds skip in place.
  - h is AllGathered between the layers inside the same NEFF.

Falls back to a pure-JAX shard_map implementation if the Bass path fails.
"""
import numpy as np

NC = 8
N = 100000
SH = 12544            # nodes per core (128-aligned)
NPAD = SH * NC        # 100352
D = 64
H = 4
ROWW = 72             # padded node-table row width (64 feat | 1.0 | pad)
WIN = SH // 128       # 98 windows per core
AW = 65               # augmented per-head width in the k' table (64 + bias col)

_state = {}


# ----------------------------------------------------------------------------
# walrus workarounds (wait-split + Tile tail drain)
# ----------------------------------------------------------------------------
def _install_patches():
    import json as _json
    import os as _os
    from concourse import bass_utils, bass2jax, tile
    from concourse.vector_clock import ScopedClock

    if getattr(bass_utils, "_gtn_patch_installed", False):
        return
    bass_utils._gtn_patch_installed = True

    MAXW = 1

    def _split_waits(bir):
        changed = False
        for fn in bir.get("functions", []):
            for bb in fn.get("blocks", []):
                insts = bb.get("instructions")
                if not insts:
                    continue
                out = []
                for inst in insts:
                    si = inst.get("sync_info") or {}
                    waits = si.get("on_wait") or []
                    if len(waits) > MAXW:
                        changed = True
                        extra, keep = waits[:-MAXW], waits[-MAXW:]
                        for n, i0 in enumerate(range(0, len(extra), MAXW)):
                            out.append({
                                "name": f"{inst['name']}_wsplit{n}",
                                "opcode": "NoOp",
                                "engine": inst.get("engine"),
                                "ins": [], "outs": [],
                                "sync_info": {"on_wait": extra[i0:i0 + MAXW],
                                              "on_update": []},
                            })
                        si = dict(si); si["on_wait"] = keep
                        inst = dict(inst); inst["sync_info"] = si
                    out.append(inst)
                bb["instructions"] = out
        return changed

    orig = bass_utils.compile_bir_kernel

    def patched(bir_json, tmpdir, neff_name="file.neff"):
        bir = _json.loads(bir_json)
        if _split_waits(bir):
            bir_json = _json.dumps(bir).encode()
        return orig(bir_json, tmpdir, neff_name=neff_name)

    bass_utils.compile_bir_kernel = patched
    bass2jax.compile_bir_kernel = patched

    def _drain_and_barrier(self, tick_clock, wait_clock):
        nop_inst = self.nc.sync.nop(nofuse=True)
        wait_clock.add_sem_waits(
            nop_inst.ins, ScopedClock({None: tick_clock.global_clock}))
        self.nc.all_engine_barrier()
        assert self.sems is not None
        popped = self.nc._tile_sem_poison_stack.pop()
        assert popped is self._sem_poison
        self.nc.clear_and_free_semaphores(list(self.sems.allocated().values()))
        self.nc.all_engine_barrier()

    tile.TileContext._drain_and_barrier = _drain_and_barrier


# ----------------------------------------------------------------------------
# host-side preprocessing
# ----------------------------------------------------------------------------
def _prep_edges(edge_index):
    src = np.asarray(edge_index[0], np.int64)
    dst = np.asarray(edge_index[1], np.int64)
    per_core = []
    t_w = 1
    for c in range(NC):
        lo = c * SH
        m = (dst >= lo) & (dst < lo + SH)
        s = src[m]
        d = dst[m] - lo
        order = np.argsort(d, kind="stable")
        s, d = s[order], d[order]
        w = d >> 7
        cnt = np.bincount(w, minlength=WIN).astype(np.int64)
        if len(s):
            t_w = max(t_w, int(np.ceil(cnt.max() / 128)))
        per_core.append((s, d, cnt))
    cols = WIN * t_w
    idx_src = np.zeros((NC, 128, cols), np.int32)
    idx_dst = np.zeros((NC, 128, cols), np.int32)
    slot = np.full((NC, 128, cols), -1.0, np.float32)
    for c, (s, d, cnt) in enumerate(per_core):
        start = 0
        for win in range(WIN):
            n = int(cnt[win])
            ss = s[start:start + n]
            dd = d[start:start + n]
            for t in range(int(np.ceil(n / 128))):
                a, b = t * 128, min((t + 1) * 128, n)
                col = win * t_w + t
                idx_src[c, 0:b - a, col] = ss[a:b]
                idx_dst[c, 0:b - a, col] = dd[a:b]
                slot[c, 0:b - a, col] = (dd[a:b] - 128 * win).astype(np.float32)
            start += n
    return idx_src, idx_dst, slot, t_w


def _prep_weights(Wq, bq, Wk, bk, Wv, bv, Ws, bs):
    Wq = np.asarray(Wq, np.float32); Wk = np.asarray(Wk, np.float32)
    Wv = np.asarray(Wv, np.float32); Ws = np.asarray(Ws, np.float32)
    bq = np.asarray(bq, np.float32); bv = np.asarray(bv, np.float32)
    bs = np.asarray(bs, np.float32)
    wkqs = np.zeros((AW, H * AW), np.float32)
    for h in range(H):
        Wq_h = Wq[:, h * D:(h + 1) * D]
        Wk_h = Wk[:, h * D:(h + 1) * D]
        wkqs[0:D, h * AW:h * AW + D] = Wk_h @ Wq_h.T
        wkqs[0:D, h * AW + D] = Wk_h @ bq[h * D:(h + 1) * D]
    wv_aug = np.concatenate([Wv, bv[None, :]], axis=0)      # [65, 256]
    ws_aug = np.concatenate([Ws, bs[None, :]], axis=0)      # [65, 64]
    return wkqs, wv_aug, ws_aug


# ----------------------------------------------------------------------------
# Bass program
# ----------------------------------------------------------------------------
def _build_nc(t_w, ablate=(), w_limit=WIN, layers=(1, 2)):
    import concourse.bass as bass
    import concourse.mybir as mybir
    import concourse.tile as tile
    from concourse.masks import make_identity

    f32 = mybir.dt.float32
    i32 = mybir.dt.int32
    COLS = WIN * t_w

    nc = bass.Bass()
    xsh = nc.dram_tensor("xsh", [SH, ROWW], f32, kind="ExternalInput")
    isrc = nc.dram_tensor("isrc", [128, COLS], i32, kind="ExternalInput")
    idst = nc.dram_tensor("idst", [128, COLS], i32, kind="ExternalInput")
    slot = nc.dram_tensor("slot", [128, COLS], f32, kind="ExternalInput")
    wk1 = nc.dram_tensor("wk1", [AW, H * AW], f32, kind="ExternalInput")
    wv1 = nc.dram_tensor("wv1", [AW, H * D], f32, kind="ExternalInput")
    ws1 = nc.dram_tensor("ws1", [AW, D], f32, kind="ExternalInput")
    wk2 = nc.dram_tensor("wk2", [AW, H * AW], f32, kind="ExternalInput")
    wv2 = nc.dram_tensor("wv2", [AW, H * D], f32, kind="ExternalInput")
    ws2 = nc.dram_tensor("ws2", [AW, D], f32, kind="ExternalInput")
    iota = nc.dram_tensor("iota", [128, 128], f32, kind="ExternalInput")
    out_sh = nc.dram_tensor("out_sh", [SH, D], f32, kind="ExternalOutput")

    xbounce = nc.dram_tensor("xbounce", [SH, ROWW], f32)
    xfull = nc.dram_tensor("xfull", [NPAD, ROWW], f32, addr_space="Shared")
    hown = nc.dram_tensor("hown", [SH, ROWW], f32)
    hbounce = nc.dram_tensor("hbounce", [SH, ROWW], f32)
    hfull = nc.dram_tensor("hfull", [NPAD, ROWW], f32, addr_space="Shared")

    groups = [list(range(NC))]

    with tile.TileContext(nc) as tc:
        with tc.tile_pool(name="const", bufs=1) as cp, \
             tc.tile_pool(name="idxp", bufs=1) as ip, \
             tc.tile_pool(name="gat", bufs=12) as gp, \
             tc.tile_pool(name="work", bufs=6) as wp, \
             tc.tile_pool(name="winp", bufs=3) as wnp, \
             tc.tile_pool(name="pmmA", bufs=2, space="PSUM") as pA, \
             tc.tile_pool(name="pmmB", bufs=2, space="PSUM") as pB, \
             tc.tile_pool(name="ptr", bufs=2, space="PSUM") as pT, \
             tc.tile_pool(name="pwin", bufs=2, space="PSUM") as pW:

            ident = cp.tile([128, 128], f32)
            make_identity(nc, ident[:])
            iota_t = cp.tile([128, 128], f32)
            nc.sync.dma_start(out=iota_t[:], in_=iota[:])

            w_k = {}
            w_v = {}
            w_s = {}
            for L, (wk, wv, ws) in ((1, (wk1, wv1, ws1)),
                                    (2, (wk2, wv2, ws2))):
                w_k[L] = cp.tile([AW, H * AW], f32, tag=f"wk{L}", name=f"wk{L}_t")
                nc.sync.dma_start(out=w_k[L][:], in_=wk[:])
                w_v[L] = cp.tile([AW, H * D], f32, tag=f"wv{L}", name=f"wv{L}_t")
                nc.sync.dma_start(out=w_v[L][:], in_=wv[:])
                w_s[L] = cp.tile([AW, D], f32, tag=f"ws{L}", name=f"ws{L}_t")
                nc.sync.dma_start(out=w_s[L][:], in_=ws[:])

            cdummy = cp.tile([128, H * AW], f32)
            nc.gpsimd.memset(cdummy[:], 0.0)
            isrc_t = ip.tile([128, COLS], i32)
            nc.sync.dma_start(out=isrc_t[:], in_=isrc[:])
            idst_t = ip.tile([128, COLS], i32)
            nc.sync.dma_start(out=idst_t[:], in_=idst[:])
            slot_t = ip.tile([128, COLS], f32)
            nc.sync.dma_start(out=slot_t[:], in_=slot[:])

            # x -> internal bounce -> allgather to the full table
            nc.gpsimd.dma_start(out=xbounce[:], in_=xsh[:])
            nc.gpsimd.collective_compute(
                "AllGather", mybir.AluOpType.bypass, replica_groups=groups,
                ins=[xbounce[:]], outs=[xfull[:]])

            def layer(L, table, own, relu, h_out):
                """One TransformerConv layer."""
                for w in range(w_limit):
                    psw = pW.tile([128, H * AW], f32, tag="pw")
                    xow = wnp.tile([128, ROWW], f32, tag="xow")
                    nc.sync.dma_start(out=xow[:],
                                      in_=own[w * 128:(w + 1) * 128, :])
                    for t in range(t_w):
                        col = w * t_w + t
                        xs = gp.tile([128, ROWW], f32, tag="xs")
                        if "nogather" in ablate:
                            nc.sync.dma_start(out=xs[:], in_=table[0:128, :])
                        else:
                            nc.gpsimd.indirect_dma_start(
                                out=xs[:], out_offset=None, in_=table[:],
                                in_offset=bass.IndirectOffsetOnAxis(
                                    ap=isrc_t[:, col:col + 1], axis=0))
                        if "sexpand" not in ablate:
                            xdg = gp.tile([128, ROWW], f32, tag="xdg")
                            nc.gpsimd.indirect_dma_start(
                                out=xdg[:], out_offset=None, in_=own[:],
                                in_offset=bass.IndirectOffsetOnAxis(
                                    ap=idst_t[:, col:col + 1], axis=0))
                        # one-hot slot matrix (also used to expand x_dst)
                        smat = wp.tile([128, 128], f32, tag="smat")
                        nc.vector.tensor_tensor(
                            out=smat[:],
                            in0=slot_t[:, col:col + 1].to_broadcast([128, 128]),
                            in1=iota_t[:], op=mybir.AluOpType.is_equal)
                        if "sexpand" not in ablate:
                            xd = xdg
                        else:
                            pstS = pT.tile([128, 128], f32, tag="pt")
                            nc.tensor.transpose(out=pstS[:], in_=smat[:],
                                                identity=ident[:])
                            smatT = wp.tile([128, 128], f32, tag="smatT")
                            nc.scalar.activation(
                                out=smatT[:], in_=pstS[:],
                                func=mybir.ActivationFunctionType.Copy)
                            pxd = pT.tile([128, AW], f32, tag="pt")
                            nc.tensor.matmul(out=pxd[:], lhsT=smatT[:],
                                             rhs=xow[:, 0:AW], start=True, stop=True)
                            xd = wp.tile([128, AW], f32, tag="xd")
                            nc.scalar.activation(
                                out=xd[:], in_=pxd[:],
                                func=mybir.ActivationFunctionType.Copy)
                        # transpose x_src features
                        pst = pT.tile([AW, 128], f32, tag="pt")
                        nc.tensor.transpose(out=pst[:], in_=xs[:, 0:AW],
                                            identity=ident[:])
                        xsT = wp.tile([AW, 128], f32, tag="xsT")
                        nc.scalar.activation(
                            out=xsT[:], in_=pst[:],
                            func=mybir.ActivationFunctionType.Copy)
                        # k' | s3 and v
                        psa = pA.tile([128, H * AW], f32, tag="pa")
                        nc.tensor.matmul(out=psa[:], lhsT=xsT[:],
                                         rhs=w_k[L][:], start=True, stop=True)
                        psb = pB.tile([128, H * D], f32, tag="pb")
                        nc.tensor.matmul(out=psb[:], lhsT=xsT[:],
                                         rhs=w_v[L][:], start=True, stop=True)
                        # alpha = sum_c k'(aug) * x_dst(aug)
                        if "nodve" in ablate:
                            nc.tensor.matmul(out=psw[:], lhsT=iota_t[:],
                                             rhs=cdummy[:],
                                             start=(t == 0), stop=(t == t_w - 1))
                            continue
                        tmp = wp.tile([128, H * AW], f32, tag="tmp")
                        xd_in = (cdummy if "dveconst" in ablate else xd)
                        nc.vector.tensor_tensor(
                            out=tmp[:].rearrange("p (h c) -> p h c", c=AW),
                            in0=psa[:].rearrange("p (h c) -> p h c", c=AW),
                            in1=xd_in[:, 0:AW].rearrange("p (o c) -> p o c", o=1)
                                .to_broadcast([128, H, AW]),
                            op=mybir.AluOpType.mult)
                        alpha = wp.tile([128, H], f32, tag="alpha")
                        nc.vector.tensor_reduce(
                            out=alpha[:],
                            in_=tmp[:].rearrange("p (h c) -> p h c", c=AW),
                            axis=mybir.AxisListType.X, op=mybir.AluOpType.add)
                        ex = wp.tile([128, H], f32, tag="ex")
                        nc.scalar.activation(
                            out=ex[:], in_=alpha[:],
                            func=mybir.ActivationFunctionType.Exp, scale=0.125)
                        # stg = [ ex*v | ex ]
                        stg = wp.tile([128, H * AW], f32, tag="stg")
                        nc.vector.tensor_tensor(
                            out=stg[:, 0:H * D].rearrange("p (h c) -> p h c", c=D),
                            in0=psb[:].rearrange("p (h c) -> p h c", c=D),
                            in1=ex[:].rearrange("p (h o) -> p h o", o=1)
                                .to_broadcast([128, H, D]),
                            op=mybir.AluOpType.mult)
                        nc.vector.tensor_copy(out=stg[:, H * D:H * AW], in_=ex[:])
                        # segment-reduce into the window accumulator
                        nc.tensor.matmul(out=psw[:], lhsT=smat[:], rhs=stg[:],
                                         start=(t == 0), stop=(t == t_w - 1))

                    # ---- window finalize ----
                    pst2 = pT.tile([AW, 128], f32, tag="pt")
                    nc.tensor.transpose(out=pst2[:], in_=xow[:, 0:AW],
                                        identity=ident[:])
                    xowT = wnp.tile([AW, 128], f32, tag="xowT")
                    nc.scalar.activation(out=xowT[:], in_=pst2[:],
                                         func=mybir.ActivationFunctionType.Copy)
                    pss = pT.tile([128, D], f32, tag="pt")
                    nc.tensor.matmul(out=pss[:], lhsT=xowT[:], rhs=w_s[L][:],
                                     start=True, stop=True)
                    # rd = 1 / (4*denom + 4e-16)
                    rdin = wnp.tile([128, H], f32, tag="rdin")
                    nc.scalar.activation(
                        out=rdin[:], in_=psw[:, H * D:H * AW],
                        func=mybir.ActivationFunctionType.Copy,
                        scale=4.0, bias=4e-16)
                    rd = wnp.tile([128, H], f32, tag="rd")
                    nc.vector.reciprocal(out=rd[:], in_=rdin[:])
                    wtmp = wnp.tile([128, H * D], f32, tag="wtmp")
                    nc.vector.tensor_tensor(
                        out=wtmp[:].rearrange("p (h c) -> p h c", c=D),
                        in0=psw[:, 0:H * D].rearrange("p (h c) -> p h c", c=D),
                        in1=rd[:].rearrange("p (h o) -> p h o", o=1)
                            .to_broadcast([128, H, D]),
                        op=mybir.AluOpType.mult)
                    hpart = wnp.tile([128, D], f32, tag="hpart")
                    nc.vector.tensor_reduce(
                        out=hpart[:],
                        in_=wtmp[:].rearrange("p (h c) -> p c h", c=D),
                        axis=mybir.AxisListType.X, op=mybir.AluOpType.add)
                    hsum = wnp.tile([128, ROWW], f32, tag="hsum")
                    nc.vector.tensor_add(out=hsum[:, 0:D], in0=hpart[:],
                                         in1=pss[:])
                    if relu:
                        nc.scalar.activation(
                            out=hsum[:, 0:D], in_=hsum[:, 0:D],
                            func=mybir.ActivationFunctionType.Relu)
                    if h_out is None:
                        nc.sync.dma_start(
                            out=out_sh[w * 128:(w + 1) * 128, :],
                            in_=hsum[:, 0:D])
                    else:
                        nc.gpsimd.memset(hsum[:, D + 1:ROWW], 0.0)
                        nc.vector.tensor_copy(out=hsum[:, D:D + 1],
                                              in_=iota_t[:, 1:2])
                        nc.sync.dma_start(
                            out=hown[w * 128:(w + 1) * 128, :], in_=hsum[:])
                        nc.sync.dma_start(
                            out=hbounce[w * 128:(w + 1) * 128, :], in_=hsum[:])

            if 1 in layers:
                layer(1, xfull, xsh, relu=True, h_out=True)
            nc.gpsimd.collective_compute(
                "AllGather", mybir.AluOpType.bypass, replica_groups=groups,
                ins=[hbounce[:]], outs=[hfull[:]])
            if 2 in layers:
                layer(2, hfull, hown, relu=False, h_out=None)
            else:
                nc.sync.dma_start(out=out_sh[0:128, :], in_=iota_t[:, 0:D])

    return nc


# ----------------------------------------------------------------------------
# SPMD runner (cached jitted executable)
# ----------------------------------------------------------------------------
class _Runner:
    def __init__(self, nc, donate=True):
        import jax
        from jax.sharding import Mesh, PartitionSpec, NamedSharding
        try:
            from jax.experimental.shard_map import shard_map
        except ImportError:
            from jax import shard_map
        from concourse.bass2jax import (_bass_exec_p, install_neuronx_cc_hook,
                                        partition_id_tensor)
        import concourse.mybir as mybir

        install_neuronx_cc_hook()
        self.jax = jax
        self.nc = nc
        partition_name = (nc.partition_id_tensor.name
                          if nc.partition_id_tensor else None)
        in_names, out_names, out_avals = [], [], []
        zero_outs = []
        for alloc in nc.m.functions[0].allocations:
            if not isinstance(alloc, mybir.MemoryLocationSet):
                continue
            name = alloc.memorylocations[0].name
            if alloc.kind == "ExternalInput":
                if name != partition_name:
                    in_names.append(name)
            elif alloc.kind == "ExternalOutput":
                shape = tuple(alloc.tensor_shape)
                dtype = mybir.dt.np(alloc.dtype)
                out_names.append(name)
                out_avals.append(jax.core.ShapedArray(shape, dtype))
                zero_outs.append(np.zeros(shape, dtype))
        self.in_names, self.out_names = in_names, out_names
        self.out_avals, self.zero_outs = out_avals, zero_outs
        n_params, n_outs = len(in_names), len(out_names)
        all_in = in_names + out_names + ([partition_name] if partition_name else [])

        def _body(*args):
            operands = list(args)
            if partition_name is not None:
                operands.append(partition_id_tensor())
            return tuple(_bass_exec_p.bind(
                *operands, out_avals=tuple(out_avals), in_names=tuple(all_in),
                out_names=tuple(out_names), lowering_input_output_aliases=(),
                sim_require_finite=False, sim_require_nnan=False, nc=nc))

        devices = jax.devices()[:NC]
        self.mesh = Mesh(np.asarray(devices), ("core",))
        self.sh = NamedSharding(self.mesh, PartitionSpec("core"))
        kwargs = dict(keep_unused=True)
        if donate:
            kwargs["donate_argnums"] = tuple(range(n_params, n_params + n_outs))
        self.donate = donate
        self.fn = jax.jit(shard_map(
            _body, mesh=self.mesh,
            in_specs=(PartitionSpec("core"),) * (n_params + n_outs),
            out_specs=(PartitionSpec("core"),) * n_outs, check_rep=False),
            **kwargs)
        self.n_params = n_params

    def device_inputs(self, in_maps):
        concat = [
            np.concatenate([np.asarray(m[name]) for m in in_maps], axis=0)
            for name in self.in_names
        ]
        return [self.jax.device_put(a, self.sh) for a in concat]

    def zeros(self):
        return [self.jax.device_put(
            np.zeros((NC * z.shape[0], *z.shape[1:]), z.dtype), self.sh)
            for z in self.zero_outs]

    def run(self, dev_inputs):
        outs = self.fn(*dev_inputs, *self.zeros())
        return [np.asarray(o) for o in outs]


# ----------------------------------------------------------------------------
# public entry
# ----------------------------------------------------------------------------
def _warmup(t_w, in_maps):
    """The first executable loaded in a process stays ~40ms/call slower on
    this axon setup; burn that slot with a windowless throwaway kernel."""
    if "warm" in _state:
        return
    nc = _build_nc(t_w, w_limit=0, layers=())
    r = _Runner(nc, donate=False)
    dev = r.device_inputs(in_maps)
    import jax
    jax.block_until_ready(r.fn(*dev, *r.zeros()))
    _state["warm"] = True


def _prepare(x, edge_index, weights):
    """Returns (runner, in_maps)."""
    idx_src, idx_dst, slot, t_w = _prep_edges(edge_index)
    key = ("bass", t_w)
    if key not in _state:
        _install_patches()
        nc = _build_nc(t_w)
        _state[key] = ("pending", nc)
    runner = _state[key]

    x = np.asarray(x, np.float32)
    xpad = np.zeros((NPAD, ROWW), np.float32)
    xpad[:N, 0:D] = x
    xpad[:, D] = 1.0

    wk1, wv1, ws1 = _prep_weights(*weights[0])
    wk2, wv2, ws2 = _prep_weights(*weights[1])
    iota = np.broadcast_to(np.arange(128, dtype=np.float32), (128, 128)).copy()

    in_maps = []
    for c in range(NC):
        in_maps.append({
            "xsh": xpad[c * SH:(c + 1) * SH],
            "isrc": idx_src[c], "idst": idx_dst[c], "slot": slot[c],
            "wk1": wk1, "wv1": wv1, "ws1": ws1,
            "wk2": wk2, "wv2": wv2, "ws2": ws2,
            "iota": iota,
        })
    if isinstance(runner, tuple):
        _warmup(t_w, in_maps)
        runner = _Runner(runner[1])
        _state[key] = runner
    return runner, in_maps


def _kernel_bass(x, edge_index, weights):
    runner, in_maps = _prepare(x, edge_index, weights)
    dev_in = runner.device_inputs(in_maps)
    outs = runner.run(dev_in)
    full = outs[0].reshape(NC * SH, D)
    return full[:N].copy()


# ---------------------------- JAX fallback ----------------------------------
def _kernel_jax(x, edge_index, weights):
    import jax
    import jax.numpy as jnp
    from jax.sharding import Mesh, NamedSharding, PartitionSpec as P
    try:
        from jax.experimental.shard_map import shard_map
    except ImportError:
        from jax import shard_map

    E = edge_index.shape[1]
    M = NC
    mesh = Mesh(np.array(jax.devices()[:M]), ('x',))
    rep = NamedSharding(mesh, P())
    esh = NamedSharding(mesh, P('x'))
    inv = np.float32(1.0 / np.sqrt(D))

    def smap(fn, in_specs, out_specs):
        return jax.jit(shard_map(fn, mesh=mesh, in_specs=in_specs,
                                 out_specs=out_specs))

    J = {}
    J['dense'] = smap(lambda x_, Wqkv, bqkv, Ws, bs:
                      tuple(jnp.split(x_ @ Wqkv + bqkv, 3, axis=1))
                      + (x_ @ Ws + bs,), (P(),) * 5, (P(), P(), P(), P()))
    J['gather'] = smap(lambda t, i: jnp.take(t, i, axis=0),
                       (P(), P('x')), P('x'))
    J['dot'] = smap(lambda a, b: (a * b).reshape(-1, H, D).sum(-1) * inv,
                    (P('x'), P('x')), P('x'))
    J['exp'] = smap(lambda a: jnp.exp(a), (P('x'),), P('x'))
    J['segsum'] = smap(lambda v, i: jax.lax.psum(
        jax.ops.segment_sum(v, i, num_segments=N), 'x'),
        (P('x'), P('x')), P())
    J['norm'] = smap(lambda ex, den, i: ex / (den[i] + 1e-16),
                     (P('x'), P(), P('x')), P('x'))
    J['msg'] = smap(lambda vs, at: vs * jnp.repeat(at, D, axis=1),
                    (P('x'), P('x')), P('x'))
    J['out'] = smap(lambda agg, skip: agg.reshape(N, H, D).mean(axis=1) + skip,
                    (P(), P()), P())
    J['relu'] = smap(lambda h: jax.nn.relu(h), (P(),), P())

    def lyr(x_d, s, d, Wqkv, bqkv, Ws, bs):
        q, k, v, skip = J['dense'](x_d, Wqkv, bqkv, Ws, bs)
        alpha = J['dot'](J['gather'](q, d), J['gather'](k, s))
        ex = J['exp'](alpha)
        den = J['segsum'](ex, d)
        attn = J['norm'](ex, den, d)
        msg = J['msg'](J['gather'](v, s), attn)
        return J['out'](J['segsum'](msg, d), skip)

    ei = np.asarray(edge_index)
    s = jax.device_put(jnp.asarray(ei[0]), esh)
    d = jax.device_put(jnp.asarray(ei[1]), esh)

    def prep(Wq, bq, Wk, bk, Wv, bv, Ws, bs):
        Wqkv = np.concatenate([Wq, Wk, Wv], axis=1)
        bqkv = np.concatenate([bq, bk, bv])
        return (jax.device_put(jnp.asarray(Wqkv), rep),
                jax.device_put(jnp.asarray(bqkv), rep),
                jax.device_put(jnp.asarray(Ws), rep),
                jax.device_put(jnp.asarray(bs), rep))

    W1 = prep(*weights[0])
    W2 = prep(*weights[1])
    x_d = jax.device_put(jnp.asarray(np.asarray(x)), rep)
    h = lyr(x_d, s, d, *W1)
    h = J['relu'](h)
    out = lyr(h, s, d, *W2)
    return np.asarray(jax.device_get(out)).astype(np.float32)


def kernel(x, edge_index, Wq1, bq1, Wk1, bk1, Wv1, bv1, Ws1, bs1,
           Wq2, bq2, Wk2, bk2, Wv2, bv2, Ws2, bs2):
    weights = ((Wq1, bq1, Wk1, bk1, Wv1, bv1, Ws1, bs1),
               (Wq2, bq2, Wk2, bk2, Wv2, bv2, Ws2, bs2))
    edge_index = np.asarray(edge_index)
    try:
        return _kernel_bass(np.asarray(x), edge_index, weights)
    except Exception as e:  # pragma: no cover - safety net
        import traceback
        traceback.print_exc()
        print(f"[kernel] bass path failed ({e!r}); falling back to JAX")
        return _kernel_jax(np.asarray(x), edge_index, weights)


# revision 20
# speedup vs baseline: 20.1250x; 5.6458x over previous
"""nn_GTN_58205396795517: 2-layer TransformerConv GNN on 8 NeuronCores.

Bass/Tile kernel. Strategy:
  - Destination-shard nodes across the 8 cores (12544 nodes/core, padded
    N=100352). Each core owns all edges into its node range; no cross-core
    reduction is needed for the segment softmax / scatter-add.
  - Per core, edges are sorted by destination and packed into 98 windows of
    128 consecutive destination nodes; each window holds a fixed number of
    128-edge tiles (capacity derived from the actual graph).
  - Attention logits use the fused bilinear form
        alpha[e,h] = x_dst . (Wk_h Wq_h^T x_src) + bq_h . (Wk_h^T x_src)
    (destination-only bias terms cancel in the segment softmax), so only
    x rows are ever gathered: 288 B per edge endpoint via indirect DMA.
  - Per tile: one-hot slot matrix (iota compare) + PE matmul performs the
    in-window segment reduction straight into PSUM; window finalize divides
    by the accumulated denominators, head-averages, and adds skip in place.
  - h is AllGathered between the layers inside the same NEFF.

Falls back to a pure-JAX shard_map implementation if the Bass path fails.
"""
import numpy as np

NC = 8
N = 100000
SH = 12544            # nodes per core (128-aligned)
NPAD = SH * NC        # 100352
D = 64
H = 4
ROWW = 72             # padded node-table row width (64 feat | 1.0 | pad)
WIN = SH // 128       # 98 windows per core
AW = 65               # augmented per-head width in the k' table (64 + bias col)

_state = {}


# ----------------------------------------------------------------------------
# walrus workarounds (wait-split + Tile tail drain)
# ----------------------------------------------------------------------------
def _install_patches():
    import json as _json
    import os as _os
    from concourse import bass_utils, bass2jax, tile
    from concourse.vector_clock import ScopedClock

    if getattr(bass_utils, "_gtn_patch_installed", False):
        return
    bass_utils._gtn_patch_installed = True

    MAXW = 1

    def _split_waits(bir):
        changed = False
        for fn in bir.get("functions", []):
            for bb in fn.get("blocks", []):
                insts = bb.get("instructions")
                if not insts:
                    continue
                out = []
                for inst in insts:
                    si = inst.get("sync_info") or {}
                    waits = si.get("on_wait") or []
                    if len(waits) > MAXW:
                        changed = True
                        extra, keep = waits[:-MAXW], waits[-MAXW:]
                        for n, i0 in enumerate(range(0, len(extra), MAXW)):
                            out.append({
                                "name": f"{inst['name']}_wsplit{n}",
                                "opcode": "NoOp",
                                "engine": inst.get("engine"),
                                "ins": [], "outs": [],
                                "sync_info": {"on_wait": extra[i0:i0 + MAXW],
                                              "on_update": []},
                            })
                        si = dict(si); si["on_wait"] = keep
                        inst = dict(inst); inst["sync_info"] = si
                    out.append(inst)
                bb["instructions"] = out
        return changed

    orig = bass_utils.compile_bir_kernel

    def patched(bir_json, tmpdir, neff_name="file.neff"):
        bir = _json.loads(bir_json)
        if _split_waits(bir):
            bir_json = _json.dumps(bir).encode()
        return orig(bir_json, tmpdir, neff_name=neff_name)

    bass_utils.compile_bir_kernel = patched
    bass2jax.compile_bir_kernel = patched

    def _drain_and_barrier(self, tick_clock, wait_clock):
        nop_inst = self.nc.sync.nop(nofuse=True)
        wait_clock.add_sem_waits(
            nop_inst.ins, ScopedClock({None: tick_clock.global_clock}))
        self.nc.sync.drain()
        self.nc.all_engine_barrier()
        assert self.sems is not None
        popped = self.nc._tile_sem_poison_stack.pop()
        assert popped is self._sem_poison
        self.nc.clear_and_free_semaphores(list(self.sems.allocated().values()))
        self.nc.all_engine_barrier()

    tile.TileContext._drain_and_barrier = _drain_and_barrier


# ----------------------------------------------------------------------------
# host-side preprocessing
# ----------------------------------------------------------------------------
def _prep_edges(edge_index):
    src = np.asarray(edge_index[0], np.int64)
    dst = np.asarray(edge_index[1], np.int64)
    per_core = []
    t_w = 1
    for c in range(NC):
        lo = c * SH
        m = (dst >= lo) & (dst < lo + SH)
        s = src[m]
        d = dst[m] - lo
        order = np.argsort(d, kind="stable")
        s, d = s[order], d[order]
        w = d >> 7
        cnt = np.bincount(w, minlength=WIN).astype(np.int64)
        if len(s):
            t_w = max(t_w, int(np.ceil(cnt.max() / 128)))
        per_core.append((s, d, cnt))
    cols = WIN * t_w
    idx_src = np.zeros((NC, 128, cols), np.int32)
    idx_dst = np.zeros((NC, 128, cols), np.int32)
    slot = np.full((NC, 128, cols), -1.0, np.float32)
    for c, (s, d, cnt) in enumerate(per_core):
        start = 0
        for win in range(WIN):
            n = int(cnt[win])
            ss = s[start:start + n]
            dd = d[start:start + n]
            for t in range(int(np.ceil(n / 128))):
                a, b = t * 128, min((t + 1) * 128, n)
                col = win * t_w + t
                idx_src[c, 0:b - a, col] = ss[a:b]
                idx_dst[c, 0:b - a, col] = dd[a:b]
                slot[c, 0:b - a, col] = (dd[a:b] - 128 * win).astype(np.float32)
            start += n
    return idx_src, idx_dst, slot, t_w


def _prep_weights(Wq, bq, Wk, bk, Wv, bv, Ws, bs):
    Wq = np.asarray(Wq, np.float32); Wk = np.asarray(Wk, np.float32)
    Wv = np.asarray(Wv, np.float32); Ws = np.asarray(Ws, np.float32)
    bq = np.asarray(bq, np.float32); bv = np.asarray(bv, np.float32)
    bs = np.asarray(bs, np.float32)
    wkqs = np.zeros((AW, H * AW), np.float32)
    for h in range(H):
        Wq_h = Wq[:, h * D:(h + 1) * D]
        Wk_h = Wk[:, h * D:(h + 1) * D]
        wkqs[0:D, h * AW:h * AW + D] = Wk_h @ Wq_h.T
        wkqs[0:D, h * AW + D] = Wk_h @ bq[h * D:(h + 1) * D]
    wv_aug = np.concatenate([Wv, bv[None, :]], axis=0)      # [65, 256]
    ws_aug = np.concatenate([Ws, bs[None, :]], axis=0)      # [65, 64]
    return wkqs, wv_aug, ws_aug


# ----------------------------------------------------------------------------
# Bass program
# ----------------------------------------------------------------------------
def _build_nc(t_w, ablate=(), w_limit=WIN, layers=(1, 2)):
    import concourse.bass as bass
    import concourse.mybir as mybir
    import concourse.tile as tile
    from concourse.masks import make_identity

    f32 = mybir.dt.float32
    i32 = mybir.dt.int32
    COLS = WIN * t_w

    nc = bass.Bass()
    xsh = nc.dram_tensor("xsh", [SH, ROWW], f32, kind="ExternalInput")
    isrc = nc.dram_tensor("isrc", [128, COLS], i32, kind="ExternalInput")
    idst = nc.dram_tensor("idst", [128, COLS], i32, kind="ExternalInput")
    slot = nc.dram_tensor("slot", [128, COLS], f32, kind="ExternalInput")
    wk1 = nc.dram_tensor("wk1", [AW, H * AW], f32, kind="ExternalInput")
    wv1 = nc.dram_tensor("wv1", [AW, H * D], f32, kind="ExternalInput")
    ws1 = nc.dram_tensor("ws1", [AW, D], f32, kind="ExternalInput")
    wk2 = nc.dram_tensor("wk2", [AW, H * AW], f32, kind="ExternalInput")
    wv2 = nc.dram_tensor("wv2", [AW, H * D], f32, kind="ExternalInput")
    ws2 = nc.dram_tensor("ws2", [AW, D], f32, kind="ExternalInput")
    iota = nc.dram_tensor("iota", [128, 128], f32, kind="ExternalInput")
    out_sh = nc.dram_tensor("out_sh", [SH, D], f32, kind="ExternalOutput")

    xbounce = nc.dram_tensor("xbounce", [SH, ROWW], f32)
    xfull = nc.dram_tensor("xfull", [NPAD, ROWW], f32, addr_space="Shared")
    hown = nc.dram_tensor("hown", [SH, ROWW], f32)
    hbounce = nc.dram_tensor("hbounce", [SH, ROWW], f32)
    hfull = nc.dram_tensor("hfull", [NPAD, ROWW], f32, addr_space="Shared")

    groups = [list(range(NC))]

    with tile.TileContext(nc) as tc:
        with tc.tile_pool(name="const", bufs=1) as cp, \
             tc.tile_pool(name="idxp", bufs=1) as ip, \
             tc.tile_pool(name="gat", bufs=12) as gp, \
             tc.tile_pool(name="work", bufs=6) as wp, \
             tc.tile_pool(name="winp", bufs=3) as wnp, \
             tc.tile_pool(name="pmmA", bufs=2, space="PSUM") as pA, \
             tc.tile_pool(name="pmmB", bufs=2, space="PSUM") as pB, \
             tc.tile_pool(name="ptr", bufs=2, space="PSUM") as pT, \
             tc.tile_pool(name="pwin", bufs=2, space="PSUM") as pW:

            ident = cp.tile([128, 128], f32)
            make_identity(nc, ident[:])
            iota_t = cp.tile([128, 128], f32)
            nc.sync.dma_start(out=iota_t[:], in_=iota[:])

            w_k = {}
            w_v = {}
            w_s = {}
            for L, (wk, wv, ws) in ((1, (wk1, wv1, ws1)),
                                    (2, (wk2, wv2, ws2))):
                w_k[L] = cp.tile([AW, H * AW], f32, tag=f"wk{L}", name=f"wk{L}_t")
                nc.sync.dma_start(out=w_k[L][:], in_=wk[:])
                w_v[L] = cp.tile([AW, H * D], f32, tag=f"wv{L}", name=f"wv{L}_t")
                nc.sync.dma_start(out=w_v[L][:], in_=wv[:])
                w_s[L] = cp.tile([AW, D], f32, tag=f"ws{L}", name=f"ws{L}_t")
                nc.sync.dma_start(out=w_s[L][:], in_=ws[:])

            cdummy = cp.tile([128, H * AW], f32)
            nc.gpsimd.memset(cdummy[:], 0.0)
            isrc_t = ip.tile([128, COLS], i32)
            nc.sync.dma_start(out=isrc_t[:], in_=isrc[:])
            idst_t = ip.tile([128, COLS], i32)
            nc.sync.dma_start(out=idst_t[:], in_=idst[:])
            slot_t = ip.tile([128, COLS], f32)
            nc.sync.dma_start(out=slot_t[:], in_=slot[:])

            # x -> internal bounce -> allgather to the full table
            nc.gpsimd.dma_start(out=xbounce[:], in_=xsh[:])
            nc.gpsimd.collective_compute(
                "AllGather", mybir.AluOpType.bypass, replica_groups=groups,
                ins=[xbounce[:]], outs=[xfull[:]])

            def layer(L, table, own, relu, h_out):
                """One TransformerConv layer."""
                for w in range(w_limit):
                    psw = pW.tile([128, H * AW], f32, tag="pw")
                    xow = wnp.tile([128, ROWW], f32, tag="xow")
                    nc.sync.dma_start(out=xow[:],
                                      in_=own[w * 128:(w + 1) * 128, :])
                    for t in range(t_w):
                        col = w * t_w + t
                        xs = gp.tile([128, ROWW], f32, tag="xs")
                        if "nogather" in ablate:
                            nc.sync.dma_start(out=xs[:], in_=table[0:128, :])
                        else:
                            nc.gpsimd.indirect_dma_start(
                                out=xs[:], out_offset=None, in_=table[:],
                                in_offset=bass.IndirectOffsetOnAxis(
                                    ap=isrc_t[:, col:col + 1], axis=0))
                        if "sexpand" not in ablate:
                            xdg = gp.tile([128, ROWW], f32, tag="xdg")
                            nc.gpsimd.indirect_dma_start(
                                out=xdg[:], out_offset=None, in_=own[:],
                                in_offset=bass.IndirectOffsetOnAxis(
                                    ap=idst_t[:, col:col + 1], axis=0))
                        # one-hot slot matrix (also used to expand x_dst)
                        smat = wp.tile([128, 128], f32, tag="smat")
                        nc.vector.tensor_tensor(
                            out=smat[:],
                            in0=slot_t[:, col:col + 1].to_broadcast([128, 128]),
                            in1=iota_t[:], op=mybir.AluOpType.is_equal)
                        if "sexpand" not in ablate:
                            xd = xdg
                        else:
                            pstS = pT.tile([128, 128], f32, tag="pt")
                            nc.tensor.transpose(out=pstS[:], in_=smat[:],
                                                identity=ident[:])
                            smatT = wp.tile([128, 128], f32, tag="smatT")
                            nc.scalar.activation(
                                out=smatT[:], in_=pstS[:],
                                func=mybir.ActivationFunctionType.Copy)
                            pxd = pT.tile([128, AW], f32, tag="pt")
                            nc.tensor.matmul(out=pxd[:], lhsT=smatT[:],
                                             rhs=xow[:, 0:AW], start=True, stop=True)
                            xd = wp.tile([128, AW], f32, tag="xd")
                            nc.scalar.activation(
                                out=xd[:], in_=pxd[:],
                                func=mybir.ActivationFunctionType.Copy)
                        # transpose x_src features
                        pst = pT.tile([AW, 128], f32, tag="pt")
                        nc.tensor.transpose(out=pst[:], in_=xs[:, 0:AW],
                                            identity=ident[:])
                        xsT = wp.tile([AW, 128], f32, tag="xsT")
                        nc.scalar.activation(
                            out=xsT[:], in_=pst[:],
                            func=mybir.ActivationFunctionType.Copy)
                        # k' | s3 and v
                        psa = pA.tile([128, H * AW], f32, tag="pa")
                        nc.tensor.matmul(out=psa[:], lhsT=xsT[:],
                                         rhs=w_k[L][:], start=True, stop=True)
                        psb = pB.tile([128, H * D], f32, tag="pb")
                        nc.tensor.matmul(out=psb[:], lhsT=xsT[:],
                                         rhs=w_v[L][:], start=True, stop=True)
                        # alpha = sum_c k'(aug) * x_dst(aug)
                        if "nodve" in ablate:
                            nc.tensor.matmul(out=psw[:], lhsT=iota_t[:],
                                             rhs=cdummy[:],
                                             start=(t == 0), stop=(t == t_w - 1))
                            continue
                        tmp = wp.tile([128, H * AW], f32, tag="tmp")
                        xd_in = (cdummy if "dveconst" in ablate else xd)
                        nc.vector.tensor_tensor(
                            out=tmp[:].rearrange("p (h c) -> p h c", c=AW),
                            in0=psa[:].rearrange("p (h c) -> p h c", c=AW),
                            in1=xd_in[:, 0:AW].rearrange("p (o c) -> p o c", o=1)
                                .to_broadcast([128, H, AW]),
                            op=mybir.AluOpType.mult)
                        alpha = wp.tile([128, H], f32, tag="alpha")
                        nc.vector.tensor_reduce(
                            out=alpha[:],
                            in_=tmp[:].rearrange("p (h c) -> p h c", c=AW),
                            axis=mybir.AxisListType.X, op=mybir.AluOpType.add)
                        ex = wp.tile([128, H], f32, tag="ex")
                        nc.scalar.activation(
                            out=ex[:], in_=alpha[:],
                            func=mybir.ActivationFunctionType.Exp, scale=0.125)
                        # stg = [ ex*v | ex ]
                        stg = wp.tile([128, H * AW], f32, tag="stg")
                        nc.vector.tensor_tensor(
                            out=stg[:, 0:H * D].rearrange("p (h c) -> p h c", c=D),
                            in0=psb[:].rearrange("p (h c) -> p h c", c=D),
                            in1=ex[:].rearrange("p (h o) -> p h o", o=1)
                                .to_broadcast([128, H, D]),
                            op=mybir.AluOpType.mult)
                        nc.vector.tensor_copy(out=stg[:, H * D:H * AW], in_=ex[:])
                        # segment-reduce into the window accumulator
                        nc.tensor.matmul(out=psw[:], lhsT=smat[:], rhs=stg[:],
                                         start=(t == 0), stop=(t == t_w - 1))

                    # ---- window finalize ----
                    pst2 = pT.tile([AW, 128], f32, tag="pt")
                    nc.tensor.transpose(out=pst2[:], in_=xow[:, 0:AW],
                                        identity=ident[:])
                    xowT = wnp.tile([AW, 128], f32, tag="xowT")
                    nc.scalar.activation(out=xowT[:], in_=pst2[:],
                                         func=mybir.ActivationFunctionType.Copy)
                    pss = pT.tile([128, D], f32, tag="pt")
                    nc.tensor.matmul(out=pss[:], lhsT=xowT[:], rhs=w_s[L][:],
                                     start=True, stop=True)
                    # rd = 1 / (4*denom + 4e-16)
                    rdin = wnp.tile([128, H], f32, tag="rdin")
                    nc.scalar.activation(
                        out=rdin[:], in_=psw[:, H * D:H * AW],
                        func=mybir.ActivationFunctionType.Copy,
                        scale=4.0, bias=4e-16)
                    rd = wnp.tile([128, H], f32, tag="rd")
                    nc.vector.reciprocal(out=rd[:], in_=rdin[:])
                    wtmp = wnp.tile([128, H * D], f32, tag="wtmp")
                    nc.vector.tensor_tensor(
                        out=wtmp[:].rearrange("p (h c) -> p h c", c=D),
                        in0=psw[:, 0:H * D].rearrange("p (h c) -> p h c", c=D),
                        in1=rd[:].rearrange("p (h o) -> p h o", o=1)
                            .to_broadcast([128, H, D]),
                        op=mybir.AluOpType.mult)
                    hpart = wnp.tile([128, D], f32, tag="hpart")
                    nc.vector.tensor_reduce(
                        out=hpart[:],
                        in_=wtmp[:].rearrange("p (h c) -> p c h", c=D),
                        axis=mybir.AxisListType.X, op=mybir.AluOpType.add)
                    hsum = wnp.tile([128, ROWW], f32, tag="hsum")
                    nc.vector.tensor_add(out=hsum[:, 0:D], in0=hpart[:],
                                         in1=pss[:])
                    if relu:
                        nc.scalar.activation(
                            out=hsum[:, 0:D], in_=hsum[:, 0:D],
                            func=mybir.ActivationFunctionType.Relu)
                    if h_out is None:
                        nc.sync.dma_start(
                            out=out_sh[w * 128:(w + 1) * 128, :],
                            in_=hsum[:, 0:D])
                    else:
                        nc.gpsimd.memset(hsum[:, D + 1:ROWW], 0.0)
                        nc.vector.tensor_copy(out=hsum[:, D:D + 1],
                                              in_=iota_t[:, 1:2])
                        nc.sync.dma_start(
                            out=hown[w * 128:(w + 1) * 128, :], in_=hsum[:])
                        nc.sync.dma_start(
                            out=hbounce[w * 128:(w + 1) * 128, :], in_=hsum[:])

            if 1 in layers:
                layer(1, xfull, xsh, relu=True, h_out=True)
            nc.gpsimd.collective_compute(
                "AllGather", mybir.AluOpType.bypass, replica_groups=groups,
                ins=[hbounce[:]], outs=[hfull[:]])
            if 2 in layers:
                layer(2, hfull, hown, relu=False, h_out=None)
            else:
                nc.sync.dma_start(out=out_sh[0:128, :], in_=iota_t[:, 0:D])

    return nc


# ----------------------------------------------------------------------------
# SPMD runner (cached jitted executable)
# ----------------------------------------------------------------------------
class _Runner:
    def __init__(self, nc, donate=True):
        import jax
        from jax.sharding import Mesh, PartitionSpec, NamedSharding
        try:
            from jax.experimental.shard_map import shard_map
        except ImportError:
            from jax import shard_map
        from concourse.bass2jax import (_bass_exec_p, install_neuronx_cc_hook,
                                        partition_id_tensor)
        import concourse.mybir as mybir

        install_neuronx_cc_hook()
        self.jax = jax
        self.nc = nc
        partition_name = (nc.partition_id_tensor.name
                          if nc.partition_id_tensor else None)
        in_names, out_names, out_avals = [], [], []
        zero_outs = []
        for alloc in nc.m.functions[0].allocations:
            if not isinstance(alloc, mybir.MemoryLocationSet):
                continue
            name = alloc.memorylocations[0].name
            if alloc.kind == "ExternalInput":
                if name != partition_name:
                    in_names.append(name)
            elif alloc.kind == "ExternalOutput":
                shape = tuple(alloc.tensor_shape)
                dtype = mybir.dt.np(alloc.dtype)
                out_names.append(name)
                out_avals.append(jax.core.ShapedArray(shape, dtype))
                zero_outs.append(np.zeros(shape, dtype))
        self.in_names, self.out_names = in_names, out_names
        self.out_avals, self.zero_outs = out_avals, zero_outs
        n_params, n_outs = len(in_names), len(out_names)
        all_in = in_names + out_names + ([partition_name] if partition_name else [])

        def _body(*args):
            operands = list(args)
            if partition_name is not None:
                operands.append(partition_id_tensor())
            return tuple(_bass_exec_p.bind(
                *operands, out_avals=tuple(out_avals), in_names=tuple(all_in),
                out_names=tuple(out_names), lowering_input_output_aliases=(),
                sim_require_finite=False, sim_require_nnan=False, nc=nc))

        devices = jax.devices()[:NC]
        self.mesh = Mesh(np.asarray(devices), ("core",))
        self.sh = NamedSharding(self.mesh, PartitionSpec("core"))
        kwargs = dict(keep_unused=True)
        if donate:
            kwargs["donate_argnums"] = tuple(range(n_params, n_params + n_outs))
        self.donate = donate
        self.fn = jax.jit(shard_map(
            _body, mesh=self.mesh,
            in_specs=(PartitionSpec("core"),) * (n_params + n_outs),
            out_specs=(PartitionSpec("core"),) * n_outs, check_rep=False),
            **kwargs)
        self.n_params = n_params
        self._compiled = None

    def compiled(self, dev_inputs, zeros):
        """AOT-compile with the bass effect suppressed (C++ fast dispatch)."""
        if self._compiled is None:
            from concourse.bass2jax import fast_dispatch_compile
            args = [self.jax.ShapeDtypeStruct(a.shape, a.dtype)
                    for a in (*dev_inputs, *zeros)]
            self._compiled = fast_dispatch_compile(
                lambda: self.fn.lower(*args).compile())
        return self._compiled

    def device_inputs(self, in_maps):
        concat = [
            np.concatenate([np.asarray(m[name]) for m in in_maps], axis=0)
            for name in self.in_names
        ]
        return [self.jax.device_put(a, self.sh) for a in concat]

    def zeros(self):
        return [self.jax.device_put(
            np.zeros((NC * z.shape[0], *z.shape[1:]), z.dtype), self.sh)
            for z in self.zero_outs]

    def run(self, dev_inputs):
        outs = self.fn(*dev_inputs, *self.zeros())
        return [np.asarray(o) for o in outs]


# ----------------------------------------------------------------------------
# public entry
# ----------------------------------------------------------------------------
def _warmup(t_w, in_maps):
    """The first executable loaded in a process stays ~40ms/call slower on
    this axon setup; burn that slot with a windowless throwaway kernel."""
    if "warm" in _state:
        return
    nc = _build_nc(t_w, w_limit=0, layers=())
    r = _Runner(nc, donate=False)
    dev = r.device_inputs(in_maps)
    import jax
    jax.block_until_ready(r.fn(*dev, *r.zeros()))
    _state["warm"] = True


def _prepare(x, edge_index, weights):
    """Returns (runner, in_maps)."""
    idx_src, idx_dst, slot, t_w = _prep_edges(edge_index)
    key = ("bass", t_w)
    if key not in _state:
        _install_patches()
        nc = _build_nc(t_w)
        _state[key] = ("pending", nc)
    runner = _state[key]

    x = np.asarray(x, np.float32)
    xpad = np.zeros((NPAD, ROWW), np.float32)
    xpad[:N, 0:D] = x
    xpad[:, D] = 1.0

    wk1, wv1, ws1 = _prep_weights(*weights[0])
    wk2, wv2, ws2 = _prep_weights(*weights[1])
    iota = np.broadcast_to(np.arange(128, dtype=np.float32), (128, 128)).copy()

    in_maps = []
    for c in range(NC):
        in_maps.append({
            "xsh": xpad[c * SH:(c + 1) * SH],
            "isrc": idx_src[c], "idst": idx_dst[c], "slot": slot[c],
            "wk1": wk1, "wv1": wv1, "ws1": ws1,
            "wk2": wk2, "wv2": wv2, "ws2": ws2,
            "iota": iota,
        })
    if isinstance(runner, tuple):
        _warmup(t_w, in_maps)
        runner = _Runner(runner[1])
        _state[key] = runner
    return runner, in_maps


def _kernel_bass(x, edge_index, weights):
    runner, in_maps = _prepare(x, edge_index, weights)
    dev_in = runner.device_inputs(in_maps)
    outs = runner.run(dev_in)
    full = outs[0].reshape(NC * SH, D)
    return full[:N].copy()


# ---------------------------- JAX fallback ----------------------------------
def _kernel_jax(x, edge_index, weights):
    import jax
    import jax.numpy as jnp
    from jax.sharding import Mesh, NamedSharding, PartitionSpec as P
    try:
        from jax.experimental.shard_map import shard_map
    except ImportError:
        from jax import shard_map

    E = edge_index.shape[1]
    M = NC
    mesh = Mesh(np.array(jax.devices()[:M]), ('x',))
    rep = NamedSharding(mesh, P())
    esh = NamedSharding(mesh, P('x'))
    inv = np.float32(1.0 / np.sqrt(D))

    def smap(fn, in_specs, out_specs):
        return jax.jit(shard_map(fn, mesh=mesh, in_specs=in_specs,
                                 out_specs=out_specs))

    J = {}
    J['dense'] = smap(lambda x_, Wqkv, bqkv, Ws, bs:
                      tuple(jnp.split(x_ @ Wqkv + bqkv, 3, axis=1))
                      + (x_ @ Ws + bs,), (P(),) * 5, (P(), P(), P(), P()))
    J['gather'] = smap(lambda t, i: jnp.take(t, i, axis=0),
                       (P(), P('x')), P('x'))
    J['dot'] = smap(lambda a, b: (a * b).reshape(-1, H, D).sum(-1) * inv,
                    (P('x'), P('x')), P('x'))
    J['exp'] = smap(lambda a: jnp.exp(a), (P('x'),), P('x'))
    J['segsum'] = smap(lambda v, i: jax.lax.psum(
        jax.ops.segment_sum(v, i, num_segments=N), 'x'),
        (P('x'), P('x')), P())
    J['norm'] = smap(lambda ex, den, i: ex / (den[i] + 1e-16),
                     (P('x'), P(), P('x')), P('x'))
    J['msg'] = smap(lambda vs, at: vs * jnp.repeat(at, D, axis=1),
                    (P('x'), P('x')), P('x'))
    J['out'] = smap(lambda agg, skip: agg.reshape(N, H, D).mean(axis=1) + skip,
                    (P(), P()), P())
    J['relu'] = smap(lambda h: jax.nn.relu(h), (P(),), P())

    def lyr(x_d, s, d, Wqkv, bqkv, Ws, bs):
        q, k, v, skip = J['dense'](x_d, Wqkv, bqkv, Ws, bs)
        alpha = J['dot'](J['gather'](q, d), J['gather'](k, s))
        ex = J['exp'](alpha)
        den = J['segsum'](ex, d)
        attn = J['norm'](ex, den, d)
        msg = J['msg'](J['gather'](v, s), attn)
        return J['out'](J['segsum'](msg, d), skip)

    ei = np.asarray(edge_index)
    s = jax.device_put(jnp.asarray(ei[0]), esh)
    d = jax.device_put(jnp.asarray(ei[1]), esh)

    def prep(Wq, bq, Wk, bk, Wv, bv, Ws, bs):
        Wqkv = np.concatenate([Wq, Wk, Wv], axis=1)
        bqkv = np.concatenate([bq, bk, bv])
        return (jax.device_put(jnp.asarray(Wqkv), rep),
                jax.device_put(jnp.asarray(bqkv), rep),
                jax.device_put(jnp.asarray(Ws), rep),
                jax.device_put(jnp.asarray(bs), rep))

    W1 = prep(*weights[0])
    W2 = prep(*weights[1])
    x_d = jax.device_put(jnp.asarray(np.asarray(x)), rep)
    h = lyr(x_d, s, d, *W1)
    h = J['relu'](h)
    out = lyr(h, s, d, *W2)
    return np.asarray(jax.device_get(out)).astype(np.float32)


def kernel(x, edge_index, Wq1, bq1, Wk1, bk1, Wv1, bv1, Ws1, bs1,
           Wq2, bq2, Wk2, bk2, Wv2, bv2, Ws2, bs2):
    weights = ((Wq1, bq1, Wk1, bk1, Wv1, bv1, Ws1, bs1),
               (Wq2, bq2, Wk2, bk2, Wv2, bv2, Ws2, bs2))
    edge_index = np.asarray(edge_index)
    try:
        return _kernel_bass(np.asarray(x), edge_index, weights)
    except Exception as e:  # pragma: no cover - safety net
        import traceback
        traceback.print_exc()
        print(f"[kernel] bass path failed ({e!r}); falling back to JAX")
        return _kernel_jax(np.asarray(x), edge_index, weights)


# revision 22
# speedup vs baseline: 23.6433x; 1.1748x over previous
"""nn_GTN_58205396795517: 2-layer TransformerConv GNN on 8 NeuronCores.

Bass/Tile kernel. Strategy:
  - Destination-shard nodes across the 8 cores (12544 nodes/core, padded
    N=100352). Each core owns all edges into its node range; no cross-core
    reduction is needed for the segment softmax / scatter-add.
  - Per core, edges are sorted by destination and packed into 98 windows of
    128 consecutive destination nodes; each window holds a fixed number of
    128-edge tiles (capacity derived from the actual graph).
  - Attention logits use the fused bilinear form
        alpha[e,h] = x_dst . (Wk_h Wq_h^T x_src) + bq_h . (Wk_h^T x_src)
    (destination-only bias terms cancel in the segment softmax), so only
    x rows are ever gathered: 288 B per edge endpoint via indirect DMA.
  - Per tile: one-hot slot matrix (iota compare) + PE matmul performs the
    in-window segment reduction straight into PSUM; window finalize divides
    by the accumulated denominators, head-averages, and adds skip in place.
  - h is AllGathered between the layers inside the same NEFF.

Falls back to a pure-JAX shard_map implementation if the Bass path fails.
"""
import numpy as np

NC = 8
N = 100000
SH = 12544            # nodes per core (128-aligned)
NPAD = SH * NC        # 100352
D = 64
H = 4
ROWW = 72             # padded node-table row width (64 feat | 1.0 | pad)
WIN = SH // 128       # 98 windows per core
AW = 65               # augmented per-head width in the k' table (64 + bias col)

_state = {}


# ----------------------------------------------------------------------------
# walrus workarounds (wait-split + Tile tail drain)
# ----------------------------------------------------------------------------
def _install_patches():
    import json as _json
    import os as _os
    from concourse import bass_utils, bass2jax, tile
    from concourse.vector_clock import ScopedClock

    if getattr(bass_utils, "_gtn_patch_installed", False):
        return
    bass_utils._gtn_patch_installed = True

    MAXW = 1

    def _split_waits(bir):
        changed = False
        for fn in bir.get("functions", []):
            for bb in fn.get("blocks", []):
                insts = bb.get("instructions")
                if not insts:
                    continue
                out = []
                for inst in insts:
                    si = inst.get("sync_info") or {}
                    waits = si.get("on_wait") or []
                    if len(waits) > MAXW:
                        changed = True
                        extra, keep = waits[:-MAXW], waits[-MAXW:]
                        for n, i0 in enumerate(range(0, len(extra), MAXW)):
                            out.append({
                                "name": f"{inst['name']}_wsplit{n}",
                                "opcode": "NoOp",
                                "engine": inst.get("engine"),
                                "ins": [], "outs": [],
                                "sync_info": {"on_wait": extra[i0:i0 + MAXW],
                                              "on_update": []},
                            })
                        si = dict(si); si["on_wait"] = keep
                        inst = dict(inst); inst["sync_info"] = si
                    out.append(inst)
                bb["instructions"] = out
        return changed

    orig = bass_utils.compile_bir_kernel

    def patched(bir_json, tmpdir, neff_name="file.neff"):
        bir = _json.loads(bir_json)
        if _split_waits(bir):
            bir_json = _json.dumps(bir).encode()
        return orig(bir_json, tmpdir, neff_name=neff_name)

    bass_utils.compile_bir_kernel = patched
    bass2jax.compile_bir_kernel = patched

    def _drain_and_barrier(self, tick_clock, wait_clock):
        nop_inst = self.nc.sync.nop(nofuse=True)
        wait_clock.add_sem_waits(
            nop_inst.ins, ScopedClock({None: tick_clock.global_clock}))
        self.nc.sync.drain()
        self.nc.all_engine_barrier()
        assert self.sems is not None
        popped = self.nc._tile_sem_poison_stack.pop()
        assert popped is self._sem_poison
        self.nc.clear_and_free_semaphores(list(self.sems.allocated().values()))
        self.nc.all_engine_barrier()

    tile.TileContext._drain_and_barrier = _drain_and_barrier


# ----------------------------------------------------------------------------
# host-side preprocessing
# ----------------------------------------------------------------------------
def _prep_edges(edge_index):
    src = np.asarray(edge_index[0], np.int64)
    dst = np.asarray(edge_index[1], np.int64)
    per_core = []
    t_w = 1
    for c in range(NC):
        lo = c * SH
        m = (dst >= lo) & (dst < lo + SH)
        s = src[m]
        d = dst[m] - lo
        order = np.argsort(d, kind="stable")
        s, d = s[order], d[order]
        w = d >> 7
        cnt = np.bincount(w, minlength=WIN).astype(np.int64)
        if len(s):
            t_w = max(t_w, int(np.ceil(cnt.max() / 128)))
        per_core.append((s, d, cnt))
    cols = WIN * t_w
    idx_src = np.zeros((NC, 128, cols), np.int32)
    idx_dst = np.zeros((NC, 128, cols), np.int32)
    slot = np.full((NC, 128, cols), -1.0, np.float32)
    for c, (s, d, cnt) in enumerate(per_core):
        start = 0
        for win in range(WIN):
            n = int(cnt[win])
            ss = s[start:start + n]
            dd = d[start:start + n]
            for t in range(int(np.ceil(n / 128))):
                a, b = t * 128, min((t + 1) * 128, n)
                col = win * t_w + t
                idx_src[c, 0:b - a, col] = ss[a:b]
                idx_dst[c, 0:b - a, col] = dd[a:b]
                slot[c, 0:b - a, col] = (dd[a:b] - 128 * win).astype(np.float32)
            start += n
    return idx_src, idx_dst, slot, t_w


def _prep_weights(Wq, bq, Wk, bk, Wv, bv, Ws, bs):
    Wq = np.asarray(Wq, np.float32); Wk = np.asarray(Wk, np.float32)
    Wv = np.asarray(Wv, np.float32); Ws = np.asarray(Ws, np.float32)
    bq = np.asarray(bq, np.float32); bv = np.asarray(bv, np.float32)
    bs = np.asarray(bs, np.float32)
    wkqs = np.zeros((AW, H * AW), np.float32)
    for h in range(H):
        Wq_h = Wq[:, h * D:(h + 1) * D]
        Wk_h = Wk[:, h * D:(h + 1) * D]
        wkqs[0:D, h * AW:h * AW + D] = Wk_h @ Wq_h.T
        wkqs[0:D, h * AW + D] = Wk_h @ bq[h * D:(h + 1) * D]
    wv_aug = np.concatenate([Wv, bv[None, :]], axis=0)      # [65, 256]
    ws_aug = np.concatenate([Ws, bs[None, :]], axis=0)      # [65, 64]
    return wkqs, wv_aug, ws_aug


# ----------------------------------------------------------------------------
# Bass program
# ----------------------------------------------------------------------------
def _build_nc(t_w, ablate=(), w_limit=WIN, layers=(1, 2), gbufs=24, wbufs=8, ptb=2, pwb=2):
    import concourse.bass as bass
    import concourse.mybir as mybir
    import concourse.tile as tile
    from concourse.masks import make_identity

    f32 = mybir.dt.float32
    i32 = mybir.dt.int32
    COLS = WIN * t_w

    nc = bass.Bass()
    xsh = nc.dram_tensor("xsh", [SH, ROWW], f32, kind="ExternalInput")
    isrc = nc.dram_tensor("isrc", [128, COLS], i32, kind="ExternalInput")
    idst = nc.dram_tensor("idst", [128, COLS], i32, kind="ExternalInput")
    slot = nc.dram_tensor("slot", [128, COLS], f32, kind="ExternalInput")
    wk1 = nc.dram_tensor("wk1", [AW, H * AW], f32, kind="ExternalInput")
    wv1 = nc.dram_tensor("wv1", [AW, H * D], f32, kind="ExternalInput")
    ws1 = nc.dram_tensor("ws1", [AW, D], f32, kind="ExternalInput")
    wk2 = nc.dram_tensor("wk2", [AW, H * AW], f32, kind="ExternalInput")
    wv2 = nc.dram_tensor("wv2", [AW, H * D], f32, kind="ExternalInput")
    ws2 = nc.dram_tensor("ws2", [AW, D], f32, kind="ExternalInput")
    iota = nc.dram_tensor("iota", [128, 128], f32, kind="ExternalInput")
    out_sh = nc.dram_tensor("out_sh", [SH, D], f32, kind="ExternalOutput")

    xbounce = nc.dram_tensor("xbounce", [SH, ROWW], f32)
    xfull = nc.dram_tensor("xfull", [NPAD, ROWW], f32, addr_space="Shared")
    hown = nc.dram_tensor("hown", [SH, ROWW], f32)
    hbounce = nc.dram_tensor("hbounce", [SH, ROWW], f32)
    hfull = nc.dram_tensor("hfull", [NPAD, ROWW], f32, addr_space="Shared")

    groups = [list(range(NC))]

    with tile.TileContext(nc) as tc:
        with tc.tile_pool(name="const", bufs=1) as cp, \
             tc.tile_pool(name="idxp", bufs=1) as ip, \
             tc.tile_pool(name="gat", bufs=gbufs) as gp, \
             tc.tile_pool(name="work", bufs=wbufs) as wp, \
             tc.tile_pool(name="winp", bufs=3) as wnp, \
             tc.tile_pool(name="pmmA", bufs=2, space="PSUM") as pA, \
             tc.tile_pool(name="pmmB", bufs=2, space="PSUM") as pB, \
             tc.tile_pool(name="ptr", bufs=ptb, space="PSUM") as pT, \
             tc.tile_pool(name="pwin", bufs=pwb, space="PSUM") as pW:

            ident = cp.tile([128, 128], f32)
            make_identity(nc, ident[:])
            iota_t = cp.tile([128, 128], f32)
            nc.sync.dma_start(out=iota_t[:], in_=iota[:])

            w_k = {}
            w_v = {}
            w_s = {}
            for L, (wk, wv, ws) in ((1, (wk1, wv1, ws1)),
                                    (2, (wk2, wv2, ws2))):
                w_k[L] = cp.tile([AW, H * AW], f32, tag=f"wk{L}", name=f"wk{L}_t")
                nc.sync.dma_start(out=w_k[L][:], in_=wk[:])
                w_v[L] = cp.tile([AW, H * D], f32, tag=f"wv{L}", name=f"wv{L}_t")
                nc.sync.dma_start(out=w_v[L][:], in_=wv[:])
                w_s[L] = cp.tile([AW, D], f32, tag=f"ws{L}", name=f"ws{L}_t")
                nc.sync.dma_start(out=w_s[L][:], in_=ws[:])

            cdummy = cp.tile([128, H * AW], f32)
            nc.gpsimd.memset(cdummy[:], 0.0)
            isrc_t = ip.tile([128, COLS], i32)
            nc.sync.dma_start(out=isrc_t[:], in_=isrc[:])
            idst_t = ip.tile([128, COLS], i32)
            nc.sync.dma_start(out=idst_t[:], in_=idst[:])
            slot_t = ip.tile([128, COLS], f32)
            nc.sync.dma_start(out=slot_t[:], in_=slot[:])

            # x -> internal bounce -> allgather to the full table
            nc.gpsimd.dma_start(out=xbounce[:], in_=xsh[:])
            nc.gpsimd.collective_compute(
                "AllGather", mybir.AluOpType.bypass, replica_groups=groups,
                ins=[xbounce[:]], outs=[xfull[:]])

            def layer(L, table, own, relu, h_out):
                """One TransformerConv layer."""
                for w in range(w_limit):
                    psw = pW.tile([128, H * AW], f32, tag="pw")
                    xow = wnp.tile([128, ROWW], f32, tag="xow")
                    nc.sync.dma_start(out=xow[:],
                                      in_=own[w * 128:(w + 1) * 128, :])
                    for t in range(t_w):
                        col = w * t_w + t
                        xs = gp.tile([128, ROWW], f32, tag="xs")
                        if "nogather" in ablate:
                            nc.sync.dma_start(out=xs[:], in_=table[0:128, :])
                        else:
                            nc.gpsimd.indirect_dma_start(
                                out=xs[:], out_offset=None, in_=table[:],
                                in_offset=bass.IndirectOffsetOnAxis(
                                    ap=isrc_t[:, col:col + 1], axis=0))
                        if "sexpand" not in ablate:
                            xdg = gp.tile([128, ROWW], f32, tag="xdg")
                            nc.gpsimd.indirect_dma_start(
                                out=xdg[:], out_offset=None, in_=own[:],
                                in_offset=bass.IndirectOffsetOnAxis(
                                    ap=idst_t[:, col:col + 1], axis=0))
                        # one-hot slot matrix (also used to expand x_dst)
                        smat = wp.tile([128, 128], f32, tag="smat")
                        nc.vector.tensor_tensor(
                            out=smat[:],
                            in0=slot_t[:, col:col + 1].to_broadcast([128, 128]),
                            in1=iota_t[:], op=mybir.AluOpType.is_equal)
                        if "sexpand" not in ablate:
                            xd = xdg
                        else:
                            pstS = pT.tile([128, 128], f32, tag="pt")
                            nc.tensor.transpose(out=pstS[:], in_=smat[:],
                                                identity=ident[:])
                            smatT = wp.tile([128, 128], f32, tag="smatT")
                            nc.scalar.activation(
                                out=smatT[:], in_=pstS[:],
                                func=mybir.ActivationFunctionType.Copy)
                            pxd = pT.tile([128, AW], f32, tag="pt")
                            nc.tensor.matmul(out=pxd[:], lhsT=smatT[:],
                                             rhs=xow[:, 0:AW], start=True, stop=True)
                            xd = wp.tile([128, AW], f32, tag="xd")
                            nc.scalar.activation(
                                out=xd[:], in_=pxd[:],
                                func=mybir.ActivationFunctionType.Copy)
                        # transpose x_src features
                        pst = pT.tile([AW, 128], f32, tag="pt")
                        nc.tensor.transpose(out=pst[:], in_=xs[:, 0:AW],
                                            identity=ident[:])
                        xsT = wp.tile([AW, 128], f32, tag="xsT")
                        nc.scalar.activation(
                            out=xsT[:], in_=pst[:],
                            func=mybir.ActivationFunctionType.Copy)
                        # k' | s3 and v
                        psa = pA.tile([128, H * AW], f32, tag="pa")
                        nc.tensor.matmul(out=psa[:], lhsT=xsT[:],
                                         rhs=w_k[L][:], start=True, stop=True)
                        psb = pB.tile([128, H * D], f32, tag="pb")
                        nc.tensor.matmul(out=psb[:], lhsT=xsT[:],
                                         rhs=w_v[L][:], start=True, stop=True)
                        # alpha = sum_c k'(aug) * x_dst(aug)
                        if "nodve" in ablate:
                            nc.tensor.matmul(out=psw[:], lhsT=iota_t[:],
                                             rhs=cdummy[:],
                                             start=(t == 0), stop=(t == t_w - 1))
                            continue
                        tmp = wp.tile([128, H * AW], f32, tag="tmp")
                        xd_in = (cdummy if "dveconst" in ablate else xd)
                        nc.vector.tensor_tensor(
                            out=tmp[:].rearrange("p (h c) -> p h c", c=AW),
                            in0=psa[:].rearrange("p (h c) -> p h c", c=AW),
                            in1=xd_in[:, 0:AW].rearrange("p (o c) -> p o c", o=1)
                                .to_broadcast([128, H, AW]),
                            op=mybir.AluOpType.mult)
                        alpha = wp.tile([128, H], f32, tag="alpha")
                        nc.vector.tensor_reduce(
                            out=alpha[:],
                            in_=tmp[:].rearrange("p (h c) -> p h c", c=AW),
                            axis=mybir.AxisListType.X, op=mybir.AluOpType.add)
                        ex = wp.tile([128, H], f32, tag="ex")
                        nc.scalar.activation(
                            out=ex[:], in_=alpha[:],
                            func=mybir.ActivationFunctionType.Exp, scale=0.125)
                        # stg = [ ex*v | ex ]
                        stg = wp.tile([128, H * AW], f32, tag="stg")
                        nc.vector.tensor_tensor(
                            out=stg[:, 0:H * D].rearrange("p (h c) -> p h c", c=D),
                            in0=psb[:].rearrange("p (h c) -> p h c", c=D),
                            in1=ex[:].rearrange("p (h o) -> p h o", o=1)
                                .to_broadcast([128, H, D]),
                            op=mybir.AluOpType.mult)
                        nc.vector.tensor_copy(out=stg[:, H * D:H * AW], in_=ex[:])
                        # segment-reduce into the window accumulator
                        nc.tensor.matmul(out=psw[:], lhsT=smat[:], rhs=stg[:],
                                         start=(t == 0), stop=(t == t_w - 1))

                    # ---- window finalize ----
                    pst2 = pT.tile([AW, 128], f32, tag="pt")
                    nc.tensor.transpose(out=pst2[:], in_=xow[:, 0:AW],
                                        identity=ident[:])
                    xowT = wnp.tile([AW, 128], f32, tag="xowT")
                    nc.scalar.activation(out=xowT[:], in_=pst2[:],
                                         func=mybir.ActivationFunctionType.Copy)
                    pss = pT.tile([128, D], f32, tag="pt")
                    nc.tensor.matmul(out=pss[:], lhsT=xowT[:], rhs=w_s[L][:],
                                     start=True, stop=True)
                    # rd = 1 / (4*denom + 4e-16)
                    rdin = wnp.tile([128, H], f32, tag="rdin")
                    nc.scalar.activation(
                        out=rdin[:], in_=psw[:, H * D:H * AW],
                        func=mybir.ActivationFunctionType.Copy,
                        scale=4.0, bias=4e-16)
                    rd = wnp.tile([128, H], f32, tag="rd")
                    nc.vector.reciprocal(out=rd[:], in_=rdin[:])
                    wtmp = wnp.tile([128, H * D], f32, tag="wtmp")
                    nc.vector.tensor_tensor(
                        out=wtmp[:].rearrange("p (h c) -> p h c", c=D),
                        in0=psw[:, 0:H * D].rearrange("p (h c) -> p h c", c=D),
                        in1=rd[:].rearrange("p (h o) -> p h o", o=1)
                            .to_broadcast([128, H, D]),
                        op=mybir.AluOpType.mult)
                    hpart = wnp.tile([128, D], f32, tag="hpart")
                    nc.vector.tensor_reduce(
                        out=hpart[:],
                        in_=wtmp[:].rearrange("p (h c) -> p c h", c=D),
                        axis=mybir.AxisListType.X, op=mybir.AluOpType.add)
                    hsum = wnp.tile([128, ROWW], f32, tag="hsum")
                    nc.vector.tensor_add(out=hsum[:, 0:D], in0=hpart[:],
                                         in1=pss[:])
                    if relu:
                        nc.scalar.activation(
                            out=hsum[:, 0:D], in_=hsum[:, 0:D],
                            func=mybir.ActivationFunctionType.Relu)
                    if h_out is None:
                        nc.sync.dma_start(
                            out=out_sh[w * 128:(w + 1) * 128, :],
                            in_=hsum[:, 0:D])
                    else:
                        nc.gpsimd.memset(hsum[:, D + 1:ROWW], 0.0)
                        nc.vector.tensor_copy(out=hsum[:, D:D + 1],
                                              in_=iota_t[:, 1:2])
                        nc.sync.dma_start(
                            out=hown[w * 128:(w + 1) * 128, :], in_=hsum[:])
                        nc.sync.dma_start(
                            out=hbounce[w * 128:(w + 1) * 128, :], in_=hsum[:])

            if 1 in layers:
                layer(1, xfull, xsh, relu=True, h_out=True)
            nc.gpsimd.collective_compute(
                "AllGather", mybir.AluOpType.bypass, replica_groups=groups,
                ins=[hbounce[:]], outs=[hfull[:]])
            if 2 in layers:
                layer(2, hfull, hown, relu=False, h_out=None)
            else:
                nc.sync.dma_start(out=out_sh[0:128, :], in_=iota_t[:, 0:D])

    return nc


# ----------------------------------------------------------------------------
# SPMD runner (cached jitted executable)
# ----------------------------------------------------------------------------
class _Runner:
    def __init__(self, nc, donate=True):
        import jax
        from jax.sharding import Mesh, PartitionSpec, NamedSharding
        try:
            from jax.experimental.shard_map import shard_map
        except ImportError:
            from jax import shard_map
        from concourse.bass2jax import (_bass_exec_p, install_neuronx_cc_hook,
                                        partition_id_tensor)
        import concourse.mybir as mybir

        install_neuronx_cc_hook()
        self.jax = jax
        self.nc = nc
        partition_name = (nc.partition_id_tensor.name
                          if nc.partition_id_tensor else None)
        in_names, out_names, out_avals = [], [], []
        zero_outs = []
        for alloc in nc.m.functions[0].allocations:
            if not isinstance(alloc, mybir.MemoryLocationSet):
                continue
            name = alloc.memorylocations[0].name
            if alloc.kind == "ExternalInput":
                if name != partition_name:
                    in_names.append(name)
            elif alloc.kind == "ExternalOutput":
                shape = tuple(alloc.tensor_shape)
                dtype = mybir.dt.np(alloc.dtype)
                out_names.append(name)
                out_avals.append(jax.core.ShapedArray(shape, dtype))
                zero_outs.append(np.zeros(shape, dtype))
        self.in_names, self.out_names = in_names, out_names
        self.out_avals, self.zero_outs = out_avals, zero_outs
        n_params, n_outs = len(in_names), len(out_names)
        all_in = in_names + out_names + ([partition_name] if partition_name else [])

        def _body(*args):
            operands = list(args)
            if partition_name is not None:
                operands.append(partition_id_tensor())
            return tuple(_bass_exec_p.bind(
                *operands, out_avals=tuple(out_avals), in_names=tuple(all_in),
                out_names=tuple(out_names), lowering_input_output_aliases=(),
                sim_require_finite=False, sim_require_nnan=False, nc=nc))

        devices = jax.devices()[:NC]
        self.mesh = Mesh(np.asarray(devices), ("core",))
        self.sh = NamedSharding(self.mesh, PartitionSpec("core"))
        kwargs = dict(keep_unused=True)
        if donate:
            kwargs["donate_argnums"] = tuple(range(n_params, n_params + n_outs))
        self.donate = donate
        self.fn = jax.jit(shard_map(
            _body, mesh=self.mesh,
            in_specs=(PartitionSpec("core"),) * (n_params + n_outs),
            out_specs=(PartitionSpec("core"),) * n_outs, check_rep=False),
            **kwargs)
        self.n_params = n_params
        self._compiled = None

    def compiled(self, dev_inputs, zeros):
        """AOT-compile with the bass effect suppressed (C++ fast dispatch)."""
        if self._compiled is None:
            from concourse.bass2jax import fast_dispatch_compile
            args = [self.jax.ShapeDtypeStruct(a.shape, a.dtype)
                    for a in (*dev_inputs, *zeros)]
            self._compiled = fast_dispatch_compile(
                lambda: self.fn.lower(*args).compile())
        return self._compiled

    def device_inputs(self, in_maps):
        concat = [
            np.concatenate([np.asarray(m[name]) for m in in_maps], axis=0)
            for name in self.in_names
        ]
        return [self.jax.device_put(a, self.sh) for a in concat]

    def zeros(self):
        return [self.jax.device_put(
            np.zeros((NC * z.shape[0], *z.shape[1:]), z.dtype), self.sh)
            for z in self.zero_outs]

    def run(self, dev_inputs):
        outs = self.fn(*dev_inputs, *self.zeros())
        return [np.asarray(o) for o in outs]


# ----------------------------------------------------------------------------
# public entry
# ----------------------------------------------------------------------------
def _warmup(t_w, in_maps):
    """The first executable loaded in a process stays ~40ms/call slower on
    this axon setup; burn that slot with a windowless throwaway kernel."""
    if "warm" in _state:
        return
    nc = _build_nc(t_w, w_limit=0, layers=())
    r = _Runner(nc, donate=False)
    dev = r.device_inputs(in_maps)
    import jax
    jax.block_until_ready(r.fn(*dev, *r.zeros()))
    _state["warm"] = True


def _prepare(x, edge_index, weights):
    """Returns (runner, in_maps)."""
    idx_src, idx_dst, slot, t_w = _prep_edges(edge_index)
    key = ("bass", t_w)
    if key not in _state:
        _install_patches()
        nc = _build_nc(t_w)
        _state[key] = ("pending", nc)
    runner = _state[key]

    x = np.asarray(x, np.float32)
    xpad = np.zeros((NPAD, ROWW), np.float32)
    xpad[:N, 0:D] = x
    xpad[:, D] = 1.0

    wk1, wv1, ws1 = _prep_weights(*weights[0])
    wk2, wv2, ws2 = _prep_weights(*weights[1])
    iota = np.broadcast_to(np.arange(128, dtype=np.float32), (128, 128)).copy()

    in_maps = []
    for c in range(NC):
        in_maps.append({
            "xsh": xpad[c * SH:(c + 1) * SH],
            "isrc": idx_src[c], "idst": idx_dst[c], "slot": slot[c],
            "wk1": wk1, "wv1": wv1, "ws1": ws1,
            "wk2": wk2, "wv2": wv2, "ws2": ws2,
            "iota": iota,
        })
    if isinstance(runner, tuple):
        _warmup(t_w, in_maps)
        runner = _Runner(runner[1])
        _state[key] = runner
    return runner, in_maps


def _kernel_bass(x, edge_index, weights):
    runner, in_maps = _prepare(x, edge_index, weights)
    dev_in = runner.device_inputs(in_maps)
    outs = runner.run(dev_in)
    full = outs[0].reshape(NC * SH, D)
    return full[:N].copy()


# ---------------------------- JAX fallback ----------------------------------
def _kernel_jax(x, edge_index, weights):
    import jax
    import jax.numpy as jnp
    from jax.sharding import Mesh, NamedSharding, PartitionSpec as P
    try:
        from jax.experimental.shard_map import shard_map
    except ImportError:
        from jax import shard_map

    E = edge_index.shape[1]
    M = NC
    mesh = Mesh(np.array(jax.devices()[:M]), ('x',))
    rep = NamedSharding(mesh, P())
    esh = NamedSharding(mesh, P('x'))
    inv = np.float32(1.0 / np.sqrt(D))

    def smap(fn, in_specs, out_specs):
        return jax.jit(shard_map(fn, mesh=mesh, in_specs=in_specs,
                                 out_specs=out_specs))

    J = {}
    J['dense'] = smap(lambda x_, Wqkv, bqkv, Ws, bs:
                      tuple(jnp.split(x_ @ Wqkv + bqkv, 3, axis=1))
                      + (x_ @ Ws + bs,), (P(),) * 5, (P(), P(), P(), P()))
    J['gather'] = smap(lambda t, i: jnp.take(t, i, axis=0),
                       (P(), P('x')), P('x'))
    J['dot'] = smap(lambda a, b: (a * b).reshape(-1, H, D).sum(-1) * inv,
                    (P('x'), P('x')), P('x'))
    J['exp'] = smap(lambda a: jnp.exp(a), (P('x'),), P('x'))
    J['segsum'] = smap(lambda v, i: jax.lax.psum(
        jax.ops.segment_sum(v, i, num_segments=N), 'x'),
        (P('x'), P('x')), P())
    J['norm'] = smap(lambda ex, den, i: ex / (den[i] + 1e-16),
                     (P('x'), P(), P('x')), P('x'))
    J['msg'] = smap(lambda vs, at: vs * jnp.repeat(at, D, axis=1),
                    (P('x'), P('x')), P('x'))
    J['out'] = smap(lambda agg, skip: agg.reshape(N, H, D).mean(axis=1) + skip,
                    (P(), P()), P())
    J['relu'] = smap(lambda h: jax.nn.relu(h), (P(),), P())

    def lyr(x_d, s, d, Wqkv, bqkv, Ws, bs):
        q, k, v, skip = J['dense'](x_d, Wqkv, bqkv, Ws, bs)
        alpha = J['dot'](J['gather'](q, d), J['gather'](k, s))
        ex = J['exp'](alpha)
        den = J['segsum'](ex, d)
        attn = J['norm'](ex, den, d)
        msg = J['msg'](J['gather'](v, s), attn)
        return J['out'](J['segsum'](msg, d), skip)

    ei = np.asarray(edge_index)
    s = jax.device_put(jnp.asarray(ei[0]), esh)
    d = jax.device_put(jnp.asarray(ei[1]), esh)

    def prep(Wq, bq, Wk, bk, Wv, bv, Ws, bs):
        Wqkv = np.concatenate([Wq, Wk, Wv], axis=1)
        bqkv = np.concatenate([bq, bk, bv])
        return (jax.device_put(jnp.asarray(Wqkv), rep),
                jax.device_put(jnp.asarray(bqkv), rep),
                jax.device_put(jnp.asarray(Ws), rep),
                jax.device_put(jnp.asarray(bs), rep))

    W1 = prep(*weights[0])
    W2 = prep(*weights[1])
    x_d = jax.device_put(jnp.asarray(np.asarray(x)), rep)
    h = lyr(x_d, s, d, *W1)
    h = J['relu'](h)
    out = lyr(h, s, d, *W2)
    return np.asarray(jax.device_get(out)).astype(np.float32)


def kernel(x, edge_index, Wq1, bq1, Wk1, bk1, Wv1, bv1, Ws1, bs1,
           Wq2, bq2, Wk2, bk2, Wv2, bv2, Ws2, bs2):
    weights = ((Wq1, bq1, Wk1, bk1, Wv1, bv1, Ws1, bs1),
               (Wq2, bq2, Wk2, bk2, Wv2, bv2, Ws2, bs2))
    edge_index = np.asarray(edge_index)
    try:
        return _kernel_bass(np.asarray(x), edge_index, weights)
    except Exception as e:  # pragma: no cover - safety net
        import traceback
        traceback.print_exc()
        print(f"[kernel] bass path failed ({e!r}); falling back to JAX")
        return _kernel_jax(np.asarray(x), edge_index, weights)
